# revision 9
# baseline (speedup 1.0000x reference)
"""Trainium2 Bass kernel for nn_NetFV (NetFV pooling head).

Strategy (pure data parallel over 8 cores, 256 batches each):
  - Host: pack x into two bf16 layouts:
      xg [NSB, 120, SB*CH*128]: per 120-row chunk, cols 0=ones, 1:61=x,
        64:124=x^2 (fully contiguous per partition -> dense DMA packets;
        ones col makes the fv matmul emit asum for free at out row 0).
      xt2 [NSB/2, 128, SB*608]: transposed (f-major) x for the logits
        matmuls; TWO superbatches per tile at partition offsets 0 and 64
        so DMAs use (nearly) all 128 partitions.
  - Device, per superbatch of 8 batches (40 chunks of 120 rows):
      logits chunk [120,8] = matmul(lhsT=XT[61,128] (FWL), rhs=Waug[61,8])
      softmax: exp / rowsum / recip / mul over the whole superbatch
      fv chunk: matmul(lhsT=Xgrp[120,128]=(1|x|x^2) (FWL), rhs=act[120,8])
      -> psum [128(=asum|fv1|pad|fv2|pad), 8] per batch -> stage
  - Finishing per 64 batches, f-on-partitions, split into 4 pipeline
    stages interleaved with later superbatches so PE never stalls:
    elementwise DVE work as [60,512] ops with const broadcast APs;
    partition reductions/broadcasts via tiny PE matmuls with ones;
    1/sqrt via ACT Ln then Exp(scale=-0.5) (same ACT table set as the
    softmax Exp -> zero ACT table switches); head as 16 accumulated
    [60,18]^T x [60,64] matmuls into out.T [18,64]; host un-transposes.
"""

import math
import sys

for _p in ("/opt/trn_rl_repo", "/opt/pypackages"):
    if _p not in sys.path:
        sys.path.append(_p)

import ml_dtypes
import numpy as np

import concourse.bacc as bacc
import concourse.bass as bass
import concourse.mybir as mybir
import concourse.tile as tile
from concourse.bass_utils import run_bass_kernel_spmd

F, M, C, OUT = 60, 600, 8, 18
B = 2048
NCORES = 8
BL = B // NCORES            # 256 batches per core
SB = 8                      # batches per superbatch
NSB = BL // SB              # 32 superbatches
FGB = 64                    # batches per finishing group
NFG = BL // FGB             # 4 finishing groups
SBPF = FGB // SB            # 8 superbatches per finishing group
CH = 5                      # chunks (of 120 rows) per batch
RP = M // CH                # 120 rows per chunk
XTW = 608                   # padded transposed row length
NG = FGB * C                # 512 finishing columns

BF16 = mybir.dt.bfloat16
F32 = mybir.dt.float32
MULT = mybir.AluOpType.mult
EPS = 1e-12

_CACHE = {}


def _build_nc():
    nc = bacc.Bacc(
        "TRN2", target_bir_lowering=False, debug=False,
        enable_asserts=False, num_devices=NCORES,
    )
    xg = nc.dram_tensor("xg", [NSB, RP, SB * CH * 128], BF16,
                        kind="ExternalInput").ap()
    xt = nc.dram_tensor("xt", [NSB // 2, 128, SB * XTW], BF16,
                        kind="ExternalInput").ap()
    waug_d = nc.dram_tensor("waug", [128, C], BF16, kind="ExternalInput").ap()
    cst_d = nc.dram_tensor("cst", [128, 112], F32, kind="ExternalInput").ap()
    hds_d = nc.dram_tensor("hds", [64, 2 * C * OUT], BF16,
                           kind="ExternalInput").ap()
    y = nc.dram_tensor("y", [NFG, OUT, FGB], F32, kind="ExternalOutput").ap()

    with tile.TileContext(nc) as tc:
        _emit(tc, y, xg, xt, waug_d, cst_d, hds_d)
    nc.compile()
    return nc


def _emit(tc, y, xg, xt, waug_d, cst_d, hds_d):
    nc = tc.nc
    from contextlib import ExitStack
    ctx = ExitStack()
    with ctx:
        cpool = ctx.enter_context(tc.tile_pool(name="cpool", bufs=1))
        xpool = ctx.enter_context(tc.tile_pool(name="xpool", bufs=6))
        tpool = ctx.enter_context(tc.tile_pool(name="tpool", bufs=3))
        spool = ctx.enter_context(tc.tile_pool(name="spool", bufs=3))
        gpool = ctx.enter_context(tc.tile_pool(name="gpool", bufs=2))
        fpool = ctx.enter_context(tc.tile_pool(name="fpool", bufs=1))
        lpsum = ctx.enter_context(tc.tile_pool(name="lpsum", bufs=3, space="PSUM"))
        fpsum = ctx.enter_context(tc.tile_pool(name="fpsum", bufs=2, space="PSUM"))
        bpsum = ctx.enter_context(tc.tile_pool(name="bpsum", bufs=1, space="PSUM"))
        rpsum = ctx.enter_context(tc.tile_pool(name="rpsum", bufs=2, space="PSUM"))

        # ---- constants ----
        waug = cpool.tile([128, C], BF16)
        nc.sync.dma_start(out=waug[:], in_=waug_d[:])
        cst = cpool.tile([128, 112], F32)
        nc.sync.dma_start(out=cst[:], in_=cst_d[:])
        hds = cpool.tile([64, 2 * C * OUT], BF16)
        nc.sync.dma_start(out=hds[:], in_=hds_d[:])
        # all f-indexed consts sit at rows 1:61 (f at row 1+f), except bcc
        # at rows 65:125; masks/ones/eps packed in spare cst columns
        k1 = cst[0:64, 0 * C:1 * C]
        w2k1 = cst[0:64, 1 * C:2 * C]
        bcc = cst[64:128, 2 * C:3 * C]
        cco = cst[0:64, 3 * C:4 * C]
        dco = cst[0:64, 4 * C:5 * C]
        maskA = cst[0:64, 40:41]           # rows 1:61 = 1 (partition-sum mask)
        eps1 = cst[0:1, 41:42]             # l2-norm epsilon (x s1^2)
        eps2 = cst[0:1, 42:43]             # l2-norm epsilon (x s2^2)
        ones_r = cst[0:1, 48:112]          # [1, 64] of ones (bcast lhsT)

        def cb(ap):  # broadcast a [64, C] const across FGB batches
            return ap.unsqueeze(1).broadcast_to([64, FGB, C])

        xtt_tiles = {}
        stage_tiles = {}
        fin_state = {}

        def prefetch(sb):
            if sb >= NSB:
                return
            t = sb // 2
            if sb % 2 == 0:
                xtt = tpool.tile([128, SB * XTW], BF16, name="xtt")
                nc.sync.dma_start(out=xtt[:], in_=xt[t])
                xtt_tiles[t] = xtt
            xgt = xpool.tile([RP, SB * CH * 128], BF16, tag="xgt", name="xgt")
            nc.sync.dma_start(out=xgt[:], in_=xg[sb])
            fin_state[("xgt", sb)] = xgt

        def emit_logits(sb):
            t = sb // 2
            po = 64 * (sb % 2)
            xtt = xtt_tiles[t]
            lp = lpsum.tile([128, SB * CH * C], F32)
            for b in range(SB):
                for c in range(CH):
                    nc.tensor.matmul(
                        lp[:, (b * CH + c) * C:(b * CH + c + 1) * C],
                        xtt[po:po + 61, b * XTW + c * RP: b * XTW + c * RP + 128],
                        waug[po:po + 61, :],
                        start=True, stop=True,
                    )
            # ---- softmax over C ----
            expt = spool.tile([RP, SB * CH * C], F32, tag="expt")
            nc.scalar.activation(
                expt[:], lp[0:RP, :], mybir.ActivationFunctionType.Exp
            )
            sums = spool.tile([RP, SB * CH], F32, tag="sums")
            nc.vector.reduce_sum(
                out=sums[:],
                in_=expt.rearrange("p (k e) -> p k e", e=C),
                axis=mybir.AxisListType.X,
            )
            rin = spool.tile([RP, SB * CH], F32, tag="rin")
            nc.vector.reciprocal(rin[:], sums[:])
            actt = spool.tile([RP, SB * CH * C], BF16, tag="actt")
            nc.vector.tensor_tensor(
                out=actt.rearrange("p (k e) -> p k e", e=C),
                in0=expt.rearrange("p (k e) -> p k e", e=C),
                in1=rin.unsqueeze(2).broadcast_to([RP, SB * CH, C]),
                op=MULT,
            )
            fin_state[("actt", sb)] = actt

        def emit_fv(sb):
            fg, s = sb // SBPF, sb % SBPF
            if s == 0:
                stage_tiles[fg] = gpool.tile([128, NG], F32, tag="stage",
                                             name="stage")
            stage = stage_tiles[fg]
            xgt = fin_state.pop(("xgt", sb))
            actt = fin_state.pop(("actt", sb))
            fp = fpsum.tile([128, SB * C], F32)
            for b in range(SB):
                for c in range(CH):
                    nc.tensor.matmul(
                        fp[:, b * C:(b + 1) * C],
                        xgt[:, (b * CH + c) * 128:(b * CH + c + 1) * 128],
                        actt[:, (b * CH + c) * C:(b * CH + c + 1) * C],
                        start=(c == 0), stop=(c == CH - 1),
                    )
            nc.vector.tensor_copy(stage[:, s * SB * C:(s + 1) * SB * C], fp[:])

        # finishing, split into 4 stages emitted ~1 superbatch apart
        def emit_fin1(fg):
            stage = stage_tiles[fg]
            asb = bpsum.tile([64, NG], F32, tag="bcast")
            nc.tensor.matmul(asb[:], ones_r[:], stage[0:1, :],
                             start=True, stop=True)
            t1 = fpool.tile([64, NG], F32, tag="t1")
            nc.vector.tensor_tensor(out=t1.rearrange("p (g e) -> p g e", e=C),
                                    in0=stage[0:64, :].rearrange(
                                        "p (g e) -> p g e", e=C),
                                    in1=cb(k1), op=MULT)
            m1 = fpool.tile([64, NG], F32, tag="m1")
            nc.vector.tensor_tensor(out=m1.rearrange("p (g e) -> p g e", e=C),
                                    in0=asb.rearrange("p (g e) -> p g e", e=C),
                                    in1=cb(w2k1), op=MULT)
            fv1f = fpool.tile([64, NG], F32, tag="fv1f")
            nc.vector.tensor_sub(fv1f[:], t1[:], m1[:])
            q1 = fpool.tile([64, NG], F32, tag="q1")
            nc.vector.tensor_mul(q1[:], fv1f[:], fv1f[:])
            u1 = fpool.tile([64, NG], F32, tag="u1")
            nc.vector.tensor_tensor(out=u1.rearrange("p (g e) -> p g e", e=C),
                                    in0=asb.rearrange("p (g e) -> p g e", e=C),
                                    in1=cb(dco), op=MULT)
            u2 = fpool.tile([64, NG], F32, tag="u2")
            nc.vector.tensor_tensor(out=u2.rearrange("p (g e) -> p g e", e=C),
                                    in0=stage[64:128, :].rearrange(
                                        "p (g e) -> p g e", e=C),
                                    in1=cb(bcc), op=MULT)
            u3 = fpool.tile([64, NG], F32, tag="u3")
            nc.vector.tensor_add(u3[:], u1[:], u2[:])
            u4 = fpool.tile([64, NG], F32, tag="u4")
            nc.vector.tensor_tensor(out=u4.rearrange("p (g e) -> p g e", e=C),
                                    in0=stage[0:64, :].rearrange(
                                        "p (g e) -> p g e", e=C),
                                    in1=cb(cco), op=MULT)
            fv2n = fpool.tile([64, NG], F32, tag="fv2n")
            nc.vector.tensor_sub(fv2n[:], u3[:], u4[:])
            q2 = fpool.tile([64, NG], F32, tag="q2")
            nc.vector.tensor_mul(q2[:], fv2n[:], fv2n[:])
            fin_state[("f1", fg)] = (fv1f, q1, fv2n, q2)

        def emit_fin2(fg):
            fv1f, q1, fv2n, q2 = fin_state.pop(("f1", fg))
            r1 = rpsum.tile([1, NG], F32, tag="r")
            nc.tensor.matmul(r1[:], maskA[:], q1[:], start=True, stop=True)
            r2 = rpsum.tile([1, NG], F32, tag="r")
            nc.tensor.matmul(r2[:], maskA[:], q2[:], start=True, stop=True)
            r2c = fpool.tile([1, FGB], F32, tag="r2c")
            nc.vector.reduce_sum(out=r2c[:],
                                 in_=r2.rearrange("p (g e) -> p g e", e=C),
                                 axis=mybir.AxisListType.X)
            lr1 = fpool.tile([1, NG], F32, tag="lr1")
            nc.scalar.activation(lr1[:], r1[:],
                                 mybir.ActivationFunctionType.Ln, bias=eps1[:])
            lr2 = fpool.tile([1, FGB], F32, tag="lr2")
            nc.scalar.activation(lr2[:], r2c[:],
                                 mybir.ActivationFunctionType.Ln, bias=eps2[:])
            nr1 = fpool.tile([1, NG], F32, tag="nr1")
            nc.scalar.activation(nr1[:], lr1[:],
                                 mybir.ActivationFunctionType.Exp, scale=-0.5)
            nr2 = fpool.tile([1, FGB], F32, tag="nr2")
            nc.scalar.activation(nr2[:], lr2[:],
                                 mybir.ActivationFunctionType.Exp, scale=-0.5)
            nr2e = fpool.tile([1, NG], F32, tag="nr2e")
            nc.vector.tensor_copy(
                nr2e.rearrange("p (g e) -> p g e", e=C),
                nr2.unsqueeze(2).broadcast_to([1, FGB, C]),
            )
            fin_state[("f2", fg)] = (fv1f, fv2n, nr1, nr2e)

        def emit_fin3(fg):
            fv1f, fv2n, nr1, nr2e = fin_state.pop(("f2", fg))
            nb1 = bpsum.tile([64, NG], F32, tag="bcast")
            nc.tensor.matmul(nb1[:], ones_r[:], nr1[:], start=True, stop=True)
            fv1n = fpool.tile([64, NG], BF16, tag="fv1n")
            nc.vector.tensor_mul(fv1n[:], fv1f[:], nb1[:])
            nb2 = bpsum.tile([64, NG], F32, tag="bcast")
            nc.tensor.matmul(nb2[:], ones_r[:], nr2e[:], start=True, stop=True)
            fv2nn = fpool.tile([64, NG], BF16, tag="fv2nn")
            nc.vector.tensor_mul(fv2nn[:], fv2n[:], nb2[:])
            fin_state[("f3", fg)] = (fv1n, fv2nn)

        def emit_fin4(fg):
            fv1n, fv2nn = fin_state.pop(("f3", fg))
            hp = rpsum.tile([OUT, FGB], F32, tag="r")
            fv1v = fv1n.rearrange("p (g e) -> p g e", e=C)
            fv2v = fv2nn.rearrange("p (g e) -> p g e", e=C)
            for ci in range(C):
                nc.tensor.matmul(
                    hp[:], hds[:, ci * OUT:(ci + 1) * OUT], fv1v[:, :, ci],
                    start=(ci == 0), stop=False,
                )
            for ci in range(C):
                nc.tensor.matmul(
                    hp[:], hds[:, (C + ci) * OUT:(C + ci + 1) * OUT],
                    fv2v[:, :, ci],
                    start=False, stop=(ci == C - 1),
                )
            yt = fpool.tile([OUT, FGB], F32, tag="yt")
            nc.scalar.copy(yt[:], hp[:])
            nc.sync.dma_start(out=y[fg], in_=yt[:])

        fins = [emit_fin1, emit_fin2, emit_fin3, emit_fin4]

        # ---- software-pipelined main loop ----
        PF = 5
        for sb in range(PF):
            prefetch(sb)
        emit_logits(0)
        emit_logits(1)
        pend = []  # (fg, next_stage_idx)
        for sb in range(NSB):
            prefetch(sb + PF)
            if sb + 2 < NSB:
                emit_logits(sb + 2)
            emit_fv(sb)
            if sb % SBPF == SBPF - 1:
                pend.append([sb // SBPF, 0])
            # advance at most one finishing stage per superbatch
            if pend and pend[0][1] < 4 and sb >= 9:
                fg, st = pend[0]
                fins[st](fg)
                pend[0][1] += 1
                if pend[0][1] == 4:
                    pend.pop(0)
                    del stage_tiles[fg]
        # drain remaining finishing stages
        while pend:
            fg, st = pend[0]
            fins[st](fg)
            pend[0][1] += 1
            if pend[0][1] == 4:
                pend.pop(0)
                del stage_tiles[fg]


def _host_prep(reshaped_input, cluster_weights, covar_weights, cluster_biases,
               cluster_weights2, hidden1_weights):
    bf = ml_dtypes.bfloat16
    x = np.ascontiguousarray(reshaped_input, dtype=np.float32)
    xb = x.astype(bf)                                   # [B*M, F]
    xq = np.square(x).astype(bf)

    # fv-side packed layout: [NCORES, NSB, RP, SB*CH, 128]
    xgp = np.zeros((NCORES, NSB, RP, SB * CH, 128), dtype=bf)
    xgp[..., 0] = bf(1.0)
    xgp[..., 1:61] = (xb.reshape(NCORES, NSB, SB * CH, RP, F)
                        .transpose(0, 1, 3, 2, 4))
    xgp[..., 65:125] = (xq.reshape(NCORES, NSB, SB * CH, RP, F)
                          .transpose(0, 1, 3, 2, 4))
    xgp = xgp.reshape(NCORES, NSB, RP, SB * CH * 128)

    # logits-side transposed layout, two superbatches per 128 partitions
    x3 = xb.reshape(NCORES, NSB, SB, M, F)
    xtp = np.zeros((NCORES, NSB // 2, 2, 64, SB, XTW), dtype=bf)
    xtp[:, :, 0, 0:F, :, 0:M] = x3[:, 0::2].transpose(0, 1, 4, 2, 3)
    xtp[:, :, 1, 0:F, :, 0:M] = x3[:, 1::2].transpose(0, 1, 4, 2, 3)
    xtp[:, :, :, F, :, 0:M] = bf(1.0)
    xtp = xtp.reshape(NCORES, NSB // 2, 128, SB * XTW)

    waug = np.concatenate(
        [cluster_weights, cluster_biases[None, :]], axis=0
    ).astype(bf)                                        # [61, 8]
    waug2 = np.zeros((128, C), dtype=bf)
    waug2[0:61] = waug
    waug2[64:125] = waug

    cw = np.square(covar_weights.astype(np.float64)) + 1e-6       # [F, C]
    w2 = cluster_weights2[0].astype(np.float64)                   # [F, C]
    # per-chain scale factors keep the l2-norm sums in ACT-Ln's good
    # range (the Ln spline misbehaves above ~1e16); exactly cancelled by
    # the normalize when eps is scaled to match.
    s1 = 1.0 / float(np.abs(1.0 / cw).max())
    s2 = 1.0 / float(np.abs(1.0 / np.square(cw)).max())
    cst = np.zeros((128, 112), dtype=np.float32)
    cst[1:61, 0 * C:1 * C] = s1 / cw
    cst[1:61, 1 * C:2 * C] = s1 * w2 / cw
    cst[65:125, 2 * C:3 * C] = s2 / np.square(cw)
    cst[1:61, 3 * C:4 * C] = s2 * 2.0 * w2 / np.square(cw)
    cst[1:61, 4 * C:5 * C] = s2 * (np.square(w2) / np.square(cw) - 1.0)
    cst[1:61, 40] = 1.0          # maskA
    cst[0, 41] = EPS * s1 * s1   # eps1
    cst[0, 42] = EPS * s2 * s2   # eps2
    cst[0, 48:112] = 1.0         # ones_r

    h = hidden1_weights.astype(np.float64)              # [2*C*F, OUT]
    h1 = h[:C * F].reshape(F, C, OUT) / math.sqrt(C)    # fold 2nd l2n of fv1
    h2 = h[C * F:].reshape(F, C, OUT)
    hds = np.zeros((64, 2 * C * OUT), dtype=bf)
    hds[1:61] = np.concatenate([h1, h2], axis=1).reshape(F, 2 * C * OUT)

    in_maps = []
    for ci in range(NCORES):
        in_maps.append({
            "xg": np.ascontiguousarray(xgp[ci]),
            "xt": np.ascontiguousarray(xtp[ci]),
            "waug": waug2,
            "cst": cst,
            "hds": hds,
        })
    return in_maps


def _get_nc():
    if "nc" not in _CACHE:
        _CACHE["nc"] = _build_nc()
    return _CACHE["nc"]


def kernel(reshaped_input, cluster_weights, covar_weights, cluster_biases,
           cluster_weights2, hidden1_weights, **_kw):
    in_maps = _host_prep(reshaped_input, cluster_weights, covar_weights,
                         cluster_biases, cluster_weights2, hidden1_weights)
    nc = _get_nc()
    res = run_bass_kernel_spmd(nc, in_maps, list(range(NCORES)))
    ys = [
        res.results[ci]["y"].transpose(0, 2, 1).reshape(BL, OUT)
        for ci in range(NCORES)
    ]
    return np.ascontiguousarray(np.concatenate(ys, axis=0), dtype=np.float32)


if __name__ == "__main__":
    rng = np.random.default_rng(0)
    fake = {
        "reshaped_input": rng.standard_normal((B * M, F), dtype=np.float32),
        "cluster_weights": rng.standard_normal((F, C)).astype(np.float32) * 0.13,
        "covar_weights": rng.standard_normal((F, C)).astype(np.float32) * 0.13,
        "cluster_biases": rng.standard_normal((C,)).astype(np.float32) * 0.13,
        "cluster_weights2": rng.standard_normal((1, F, C)).astype(np.float32) * 0.13,
        "hidden1_weights": rng.standard_normal((2 * C * F, OUT)).astype(np.float32) * 0.35,
    }
    out = kernel(**fake)
    print("kernel output", out.shape, out.dtype, np.abs(out).mean())


# revision 10
# speedup vs baseline: 1.1300x; 1.1300x over previous
"""Trainium2 Bass kernel for nn_NetFV (NetFV pooling head).

Strategy (pure data parallel over 8 cores, 256 batches each):
  - Host: pack x into two bf16 layouts:
      xg [NSB, 120, SB*CH*128]: per 120-row chunk, cols 0=ones, 1:61=x,
        64:124=x^2 (fully contiguous per partition -> dense DMA packets;
        ones col makes the fv matmul emit asum for free at out row 0).
      xt2 [NSB/2, 128, SB*608]: transposed (f-major) x for the logits
        matmuls; TWO superbatches per tile at partition offsets 0 and 64
        so DMAs use (nearly) all 128 partitions.
  - Device, per superbatch of 8 batches (40 chunks of 120 rows):
      logits chunk [120,8] = matmul(lhsT=XT[61,128] (FWL), rhs=Waug[61,8])
      softmax: exp / rowsum / recip / mul over the whole superbatch
      fv chunk: matmul(lhsT=Xgrp[120,128]=(1|x|x^2) (FWL), rhs=act[120,8])
      -> psum [128(=asum|fv1|pad|fv2|pad), 8] per batch -> stage
  - Finishing per 64 batches, f-on-partitions, split into 4 pipeline
    stages interleaved with later superbatches so PE never stalls:
    elementwise DVE work as [60,512] ops with const broadcast APs;
    partition reductions/broadcasts via tiny PE matmuls with ones;
    1/sqrt via ACT Ln then Exp(scale=-0.5) (same ACT table set as the
    softmax Exp -> zero ACT table switches); head as 16 accumulated
    [60,18]^T x [60,64] matmuls into out.T [18,64]; host un-transposes.
"""

import math
import sys

for _p in ("/opt/trn_rl_repo", "/opt/pypackages"):
    if _p not in sys.path:
        sys.path.append(_p)

import ml_dtypes
import numpy as np

import concourse.bacc as bacc
import concourse.bass as bass
import concourse.mybir as mybir
import concourse.tile as tile
from concourse.bass_utils import run_bass_kernel_spmd

F, M, C, OUT = 60, 600, 8, 18
B = 2048
NCORES = 8
BL = B // NCORES            # 256 batches per core
SB = 8                      # batches per superbatch
NSB = BL // SB              # 32 superbatches
FGB = 64                    # batches per finishing group
NFG = BL // FGB             # 4 finishing groups
SBPF = FGB // SB            # 8 superbatches per finishing group
CH = 5                      # chunks (of 120 rows) per batch
RP = M // CH                # 120 rows per chunk
XTW = 608                   # padded transposed row length
NG = FGB * C                # 512 finishing columns

BF16 = mybir.dt.bfloat16
F32 = mybir.dt.float32
MULT = mybir.AluOpType.mult
EPS = 1e-12

_CACHE = {}


def _build_nc():
    nc = bacc.Bacc(
        "TRN2", target_bir_lowering=False, debug=False,
        enable_asserts=False, num_devices=NCORES,
    )
    xg = nc.dram_tensor("xg", [NSB, RP, SB * CH * 64], BF16,
                        kind="ExternalInput").ap()
    xt = nc.dram_tensor("xt", [NSB // 2, 128, SB * XTW], BF16,
                        kind="ExternalInput").ap()
    waug_d = nc.dram_tensor("waug", [128, C], BF16, kind="ExternalInput").ap()
    cst_d = nc.dram_tensor("cst", [128, 112], F32, kind="ExternalInput").ap()
    hds_d = nc.dram_tensor("hds", [64, 2 * C * OUT], BF16,
                           kind="ExternalInput").ap()
    y = nc.dram_tensor("y", [NFG, OUT, FGB], F32, kind="ExternalOutput").ap()

    with tile.TileContext(nc) as tc:
        _emit(tc, y, xg, xt, waug_d, cst_d, hds_d)
    nc.compile()
    return nc


def _emit(tc, y, xg, xt, waug_d, cst_d, hds_d):
    nc = tc.nc
    from contextlib import ExitStack
    ctx = ExitStack()
    with ctx:
        cpool = ctx.enter_context(tc.tile_pool(name="cpool", bufs=1))
        xpool = ctx.enter_context(tc.tile_pool(name="xpool", bufs=6))
        qpool = ctx.enter_context(tc.tile_pool(name="qpool", bufs=3))
        tpool = ctx.enter_context(tc.tile_pool(name="tpool", bufs=3))
        spool = ctx.enter_context(tc.tile_pool(name="spool", bufs=3))
        gpool = ctx.enter_context(tc.tile_pool(name="gpool", bufs=2))
        fpool = ctx.enter_context(tc.tile_pool(name="fpool", bufs=1))
        lpsum = ctx.enter_context(tc.tile_pool(name="lpsum", bufs=3, space="PSUM"))
        fpsum = ctx.enter_context(tc.tile_pool(name="fpsum", bufs=2, space="PSUM"))
        bpsum = ctx.enter_context(tc.tile_pool(name="bpsum", bufs=1, space="PSUM"))
        rpsum = ctx.enter_context(tc.tile_pool(name="rpsum", bufs=2, space="PSUM"))

        # ---- constants ----
        waug = cpool.tile([128, C], BF16)
        nc.sync.dma_start(out=waug[:], in_=waug_d[:])
        cst = cpool.tile([128, 112], F32)
        nc.sync.dma_start(out=cst[:], in_=cst_d[:])
        hds = cpool.tile([64, 2 * C * OUT], BF16)
        nc.sync.dma_start(out=hds[:], in_=hds_d[:])
        # all f-indexed consts sit at rows 1:61 (f at row 1+f), except bcc
        # at rows 65:125; masks/ones/eps packed in spare cst columns
        k1 = cst[0:64, 0 * C:1 * C]
        w2k1 = cst[0:64, 1 * C:2 * C]
        bcc = cst[64:128, 2 * C:3 * C]
        cco = cst[0:64, 3 * C:4 * C]
        dco = cst[0:64, 4 * C:5 * C]
        maskA = cst[0:64, 40:41]           # rows 1:61 = 1 (partition-sum mask)
        eps1 = cst[0:1, 41:42]             # l2-norm epsilon (x s1^2)
        eps2 = cst[0:1, 42:43]             # l2-norm epsilon (x s2^2)
        ones_r = cst[0:1, 48:112]          # [1, 64] of ones (bcast lhsT)

        def cb(ap):  # broadcast a [64, C] const across FGB batches
            return ap.unsqueeze(1).broadcast_to([64, FGB, C])

        xtt_tiles = {}
        stage_tiles = {}
        fin_state = {}

        def prefetch(sb):
            if sb >= NSB:
                return
            t = sb // 2
            if sb % 2 == 0:
                xtt = tpool.tile([128, SB * XTW], BF16, name="xtt")
                nc.sync.dma_start(out=xtt[:], in_=xt[t])
                xtt_tiles[t] = xtt
            xgt = xpool.tile([RP, SB * CH * 64], BF16, tag="xgt", name="xgt")
            nc.sync.dma_start(out=xgt[:], in_=xg[sb])
            fin_state[("xgt", sb)] = xgt

        def emit_logits(sb):
            t = sb // 2
            po = 64 * (sb % 2)
            xtt = xtt_tiles[t]
            xgt = fin_state[("xgt", sb)]
            xsq = qpool.tile([RP, SB * CH * 64], BF16, tag="xsq", name="xsq")
            nc.vector.tensor_mul(xsq[:], xgt[:], xgt[:])
            fin_state[("xsq", sb)] = xsq
            lp = lpsum.tile([128, SB * CH * C], F32)
            for b in range(SB):
                for c in range(CH):
                    nc.tensor.matmul(
                        lp[:, (b * CH + c) * C:(b * CH + c + 1) * C],
                        xtt[po:po + 61, b * XTW + c * RP: b * XTW + c * RP + 128],
                        waug[po:po + 61, :],
                        start=True, stop=True,
                    )
            # ---- softmax over C ----
            expt = spool.tile([RP, SB * CH * C], F32, tag="expt")
            nc.scalar.activation(
                expt[:], lp[0:RP, :], mybir.ActivationFunctionType.Exp
            )
            sums = spool.tile([RP, SB * CH], F32, tag="sums")
            nc.vector.reduce_sum(
                out=sums[:],
                in_=expt.rearrange("p (k e) -> p k e", e=C),
                axis=mybir.AxisListType.X,
            )
            rin = spool.tile([RP, SB * CH], F32, tag="rin")
            nc.vector.reciprocal(rin[:], sums[:])
            actt = spool.tile([RP, SB * CH * C], BF16, tag="actt")
            nc.vector.tensor_tensor(
                out=actt.rearrange("p (k e) -> p k e", e=C),
                in0=expt.rearrange("p (k e) -> p k e", e=C),
                in1=rin.unsqueeze(2).broadcast_to([RP, SB * CH, C]),
                op=MULT,
            )
            fin_state[("actt", sb)] = actt

        def emit_fv(sb):
            fg, s = sb // SBPF, sb % SBPF
            if s == 0:
                stage_tiles[fg] = gpool.tile([128, NG], F32, tag="stage",
                                             name="stage")
            stage = stage_tiles[fg]
            xgt = fin_state.pop(("xgt", sb))
            xsq = fin_state.pop(("xsq", sb))
            actt = fin_state.pop(("actt", sb))
            fp = fpsum.tile([128, SB * C], F32)
            for b in range(SB):
                for c in range(CH):
                    k = b * CH + c
                    nc.tensor.matmul(
                        fp[0:64, b * C:(b + 1) * C],
                        xgt[:, k * 64:(k + 1) * 64],
                        actt[:, k * C:(k + 1) * C],
                        start=(c == 0), stop=(c == CH - 1),
                    )
                    nc.tensor.matmul(
                        fp[64:128, b * C:(b + 1) * C],
                        xsq[:, k * 64:(k + 1) * 64],
                        actt[:, k * C:(k + 1) * C],
                        start=(c == 0), stop=(c == CH - 1),
                        tile_position=(0, 64),
                    )
            nc.vector.tensor_copy(stage[:, s * SB * C:(s + 1) * SB * C], fp[:])

        # finishing, split into 4 stages emitted ~1 superbatch apart
        def emit_fin1(fg):
            stage = stage_tiles[fg]
            asb = bpsum.tile([64, NG], F32, tag="bcast")
            nc.tensor.matmul(asb[:], ones_r[:], stage[0:1, :],
                             start=True, stop=True)
            t1 = fpool.tile([64, NG], F32, tag="t1")
            nc.vector.tensor_tensor(out=t1.rearrange("p (g e) -> p g e", e=C),
                                    in0=stage[0:64, :].rearrange(
                                        "p (g e) -> p g e", e=C),
                                    in1=cb(k1), op=MULT)
            m1 = fpool.tile([64, NG], F32, tag="m1")
            nc.vector.tensor_tensor(out=m1.rearrange("p (g e) -> p g e", e=C),
                                    in0=asb.rearrange("p (g e) -> p g e", e=C),
                                    in1=cb(w2k1), op=MULT)
            fv1f = fpool.tile([64, NG], F32, tag="fv1f")
            nc.vector.tensor_sub(fv1f[:], t1[:], m1[:])
            q1 = fpool.tile([64, NG], F32, tag="q1")
            nc.vector.tensor_mul(q1[:], fv1f[:], fv1f[:])
            u1 = fpool.tile([64, NG], F32, tag="u1")
            nc.vector.tensor_tensor(out=u1.rearrange("p (g e) -> p g e", e=C),
                                    in0=asb.rearrange("p (g e) -> p g e", e=C),
                                    in1=cb(dco), op=MULT)
            u2 = fpool.tile([64, NG], F32, tag="u2")
            nc.vector.tensor_tensor(out=u2.rearrange("p (g e) -> p g e", e=C),
                                    in0=stage[64:128, :].rearrange(
                                        "p (g e) -> p g e", e=C),
                                    in1=cb(bcc), op=MULT)
            u3 = fpool.tile([64, NG], F32, tag="u3")
            nc.vector.tensor_add(u3[:], u1[:], u2[:])
            u4 = fpool.tile([64, NG], F32, tag="u4")
            nc.vector.tensor_tensor(out=u4.rearrange("p (g e) -> p g e", e=C),
                                    in0=stage[0:64, :].rearrange(
                                        "p (g e) -> p g e", e=C),
                                    in1=cb(cco), op=MULT)
            fv2n = fpool.tile([64, NG], F32, tag="fv2n")
            nc.vector.tensor_sub(fv2n[:], u3[:], u4[:])
            q2 = fpool.tile([64, NG], F32, tag="q2")
            nc.vector.tensor_mul(q2[:], fv2n[:], fv2n[:])
            fin_state[("f1", fg)] = (fv1f, q1, fv2n, q2)

        def emit_fin2(fg):
            fv1f, q1, fv2n, q2 = fin_state.pop(("f1", fg))
            r1 = rpsum.tile([1, NG], F32, tag="r")
            nc.tensor.matmul(r1[:], maskA[:], q1[:], start=True, stop=True)
            r2 = rpsum.tile([1, NG], F32, tag="r")
            nc.tensor.matmul(r2[:], maskA[:], q2[:], start=True, stop=True)
            r2c = fpool.tile([1, FGB], F32, tag="r2c")
            nc.vector.reduce_sum(out=r2c[:],
                                 in_=r2.rearrange("p (g e) -> p g e", e=C),
                                 axis=mybir.AxisListType.X)
            lr1 = fpool.tile([1, NG], F32, tag="lr1")
            nc.scalar.activation(lr1[:], r1[:],
                                 mybir.ActivationFunctionType.Ln, bias=eps1[:])
            lr2 = fpool.tile([1, FGB], F32, tag="lr2")
            nc.scalar.activation(lr2[:], r2c[:],
                                 mybir.ActivationFunctionType.Ln, bias=eps2[:])
            nr1 = fpool.tile([1, NG], F32, tag="nr1")
            nc.scalar.activation(nr1[:], lr1[:],
                                 mybir.ActivationFunctionType.Exp, scale=-0.5)
            nr2 = fpool.tile([1, FGB], F32, tag="nr2")
            nc.scalar.activation(nr2[:], lr2[:],
                                 mybir.ActivationFunctionType.Exp, scale=-0.5)
            nr2e = fpool.tile([1, NG], F32, tag="nr2e")
            nc.vector.tensor_copy(
                nr2e.rearrange("p (g e) -> p g e", e=C),
                nr2.unsqueeze(2).broadcast_to([1, FGB, C]),
            )
            fin_state[("f2", fg)] = (fv1f, fv2n, nr1, nr2e)

        def emit_fin3(fg):
            fv1f, fv2n, nr1, nr2e = fin_state.pop(("f2", fg))
            nb1 = bpsum.tile([64, NG], F32, tag="bcast")
            nc.tensor.matmul(nb1[:], ones_r[:], nr1[:], start=True, stop=True)
            fv1n = fpool.tile([64, NG], BF16, tag="fv1n")
            nc.vector.tensor_mul(fv1n[:], fv1f[:], nb1[:])
            nb2 = bpsum.tile([64, NG], F32, tag="bcast")
            nc.tensor.matmul(nb2[:], ones_r[:], nr2e[:], start=True, stop=True)
            fv2nn = fpool.tile([64, NG], BF16, tag="fv2nn")
            nc.vector.tensor_mul(fv2nn[:], fv2n[:], nb2[:])
            fin_state[("f3", fg)] = (fv1n, fv2nn)

        def emit_fin4(fg):
            fv1n, fv2nn = fin_state.pop(("f3", fg))
            hp = rpsum.tile([OUT, FGB], F32, tag="r")
            fv1v = fv1n.rearrange("p (g e) -> p g e", e=C)
            fv2v = fv2nn.rearrange("p (g e) -> p g e", e=C)
            for ci in range(C):
                nc.tensor.matmul(
                    hp[:], hds[:, ci * OUT:(ci + 1) * OUT], fv1v[:, :, ci],
                    start=(ci == 0), stop=False,
                )
            for ci in range(C):
                nc.tensor.matmul(
                    hp[:], hds[:, (C + ci) * OUT:(C + ci + 1) * OUT],
                    fv2v[:, :, ci],
                    start=False, stop=(ci == C - 1),
                )
            yt = fpool.tile([OUT, FGB], F32, tag="yt")
            nc.scalar.copy(yt[:], hp[:])
            nc.sync.dma_start(out=y[fg], in_=yt[:])

        fins = [emit_fin1, emit_fin2, emit_fin3, emit_fin4]

        # ---- software-pipelined main loop ----
        PF = 5
        for sb in range(PF):
            prefetch(sb)
        emit_logits(0)
        emit_logits(1)
        pend = []  # (fg, next_stage_idx)
        for sb in range(NSB):
            prefetch(sb + PF)
            if sb + 2 < NSB:
                emit_logits(sb + 2)
            emit_fv(sb)
            if sb % SBPF == SBPF - 1:
                pend.append([sb // SBPF, 0])
            # advance at most one finishing stage per superbatch
            if pend and pend[0][1] < 4 and sb >= 9:
                fg, st = pend[0]
                fins[st](fg)
                pend[0][1] += 1
                if pend[0][1] == 4:
                    pend.pop(0)
                    del stage_tiles[fg]
        # drain remaining finishing stages
        while pend:
            fg, st = pend[0]
            fins[st](fg)
            pend[0][1] += 1
            if pend[0][1] == 4:
                pend.pop(0)
                del stage_tiles[fg]


def _host_prep(reshaped_input, cluster_weights, covar_weights, cluster_biases,
               cluster_weights2, hidden1_weights):
    bf = ml_dtypes.bfloat16
    x = np.ascontiguousarray(reshaped_input, dtype=np.float32)
    xb = x.astype(bf)                                   # [B*M, F]

    # fv-side packed layout: [NCORES, NSB, RP, SB*CH, 64]; x^2 is
    # squared on-chip (DVE) from this same tile
    xgp = np.zeros((NCORES, NSB, RP, SB * CH, 64), dtype=bf)
    xgp[..., 0] = bf(1.0)
    xgp[..., 1:61] = (xb.reshape(NCORES, NSB, SB * CH, RP, F)
                        .transpose(0, 1, 3, 2, 4))
    xgp = xgp.reshape(NCORES, NSB, RP, SB * CH * 64)

    # logits-side transposed layout, two superbatches per 128 partitions
    x3 = xb.reshape(NCORES, NSB, SB, M, F)
    xtp = np.zeros((NCORES, NSB // 2, 2, 64, SB, XTW), dtype=bf)
    xtp[:, :, 0, 0:F, :, 0:M] = x3[:, 0::2].transpose(0, 1, 4, 2, 3)
    xtp[:, :, 1, 0:F, :, 0:M] = x3[:, 1::2].transpose(0, 1, 4, 2, 3)
    xtp[:, :, :, F, :, 0:M] = bf(1.0)
    xtp = xtp.reshape(NCORES, NSB // 2, 128, SB * XTW)

    waug = np.concatenate(
        [cluster_weights, cluster_biases[None, :]], axis=0
    ).astype(bf)                                        # [61, 8]
    waug2 = np.zeros((128, C), dtype=bf)
    waug2[0:61] = waug
    waug2[64:125] = waug

    cw = np.square(covar_weights.astype(np.float64)) + 1e-6       # [F, C]
    w2 = cluster_weights2[0].astype(np.float64)                   # [F, C]
    # per-chain scale factors keep the l2-norm sums in ACT-Ln's good
    # range (the Ln spline misbehaves above ~1e16); exactly cancelled by
    # the normalize when eps is scaled to match.
    s1 = 1.0 / float(np.abs(1.0 / cw).max())
    s2 = 1.0 / float(np.abs(1.0 / np.square(cw)).max())
    cst = np.zeros((128, 112), dtype=np.float32)
    cst[1:61, 0 * C:1 * C] = s1 / cw
    cst[1:61, 1 * C:2 * C] = s1 * w2 / cw
    cst[65:125, 2 * C:3 * C] = s2 / np.square(cw)
    cst[1:61, 3 * C:4 * C] = s2 * 2.0 * w2 / np.square(cw)
    cst[1:61, 4 * C:5 * C] = s2 * (np.square(w2) / np.square(cw) - 1.0)
    cst[1:61, 40] = 1.0          # maskA
    cst[0, 41] = EPS * s1 * s1   # eps1
    cst[0, 42] = EPS * s2 * s2   # eps2
    cst[0, 48:112] = 1.0         # ones_r

    h = hidden1_weights.astype(np.float64)              # [2*C*F, OUT]
    h1 = h[:C * F].reshape(F, C, OUT) / math.sqrt(C)    # fold 2nd l2n of fv1
    h2 = h[C * F:].reshape(F, C, OUT)
    hds = np.zeros((64, 2 * C * OUT), dtype=bf)
    hds[1:61] = np.concatenate([h1, h2], axis=1).reshape(F, 2 * C * OUT)

    in_maps = []
    for ci in range(NCORES):
        in_maps.append({
            "xg": np.ascontiguousarray(xgp[ci]),
            "xt": np.ascontiguousarray(xtp[ci]),
            "waug": waug2,
            "cst": cst,
            "hds": hds,
        })
    return in_maps


def _get_nc():
    if "nc" not in _CACHE:
        _CACHE["nc"] = _build_nc()
    return _CACHE["nc"]


def kernel(reshaped_input, cluster_weights, covar_weights, cluster_biases,
           cluster_weights2, hidden1_weights, **_kw):
    in_maps = _host_prep(reshaped_input, cluster_weights, covar_weights,
                         cluster_biases, cluster_weights2, hidden1_weights)
    nc = _get_nc()
    res = run_bass_kernel_spmd(nc, in_maps, list(range(NCORES)))
    ys = [
        res.results[ci]["y"].transpose(0, 2, 1).reshape(BL, OUT)
        for ci in range(NCORES)
    ]
    return np.ascontiguousarray(np.concatenate(ys, axis=0), dtype=np.float32)


if __name__ == "__main__":
    rng = np.random.default_rng(0)
    fake = {
        "reshaped_input": rng.standard_normal((B * M, F), dtype=np.float32),
        "cluster_weights": rng.standard_normal((F, C)).astype(np.float32) * 0.13,
        "covar_weights": rng.standard_normal((F, C)).astype(np.float32) * 0.13,
        "cluster_biases": rng.standard_normal((C,)).astype(np.float32) * 0.13,
        "cluster_weights2": rng.standard_normal((1, F, C)).astype(np.float32) * 0.13,
        "hidden1_weights": rng.standard_normal((2 * C * F, OUT)).astype(np.float32) * 0.35,
    }
    out = kernel(**fake)
    print("kernel output", out.shape, out.dtype, np.abs(out).mean())


# revision 11
# speedup vs baseline: 1.1369x; 1.0061x over previous
"""Trainium2 Bass kernel for nn_NetFV (NetFV pooling head).

Strategy (pure data parallel over 8 cores, 256 batches each):
  - Host: pack x into two bf16 layouts:
      xg [NSB, 120, SB*CH*128]: per 120-row chunk, cols 0=ones, 1:61=x,
        64:124=x^2 (fully contiguous per partition -> dense DMA packets;
        ones col makes the fv matmul emit asum for free at out row 0).
      xt2 [NSB/2, 128, SB*608]: transposed (f-major) x for the logits
        matmuls; TWO superbatches per tile at partition offsets 0 and 64
        so DMAs use (nearly) all 128 partitions.
  - Device, per superbatch of 8 batches (40 chunks of 120 rows):
      logits chunk [120,8] = matmul(lhsT=XT[61,128] (FWL), rhs=Waug[61,8])
      softmax: exp / rowsum / recip / mul over the whole superbatch
      fv chunk: matmul(lhsT=Xgrp[120,128]=(1|x|x^2) (FWL), rhs=act[120,8])
      -> psum [128(=asum|fv1|pad|fv2|pad), 8] per batch -> stage
  - Finishing per 64 batches, f-on-partitions, split into 4 pipeline
    stages interleaved with later superbatches so PE never stalls:
    elementwise DVE work as [60,512] ops with const broadcast APs;
    partition reductions/broadcasts via tiny PE matmuls with ones;
    1/sqrt via ACT Ln then Exp(scale=-0.5) (same ACT table set as the
    softmax Exp -> zero ACT table switches); head as 16 accumulated
    [60,18]^T x [60,64] matmuls into out.T [18,64]; host un-transposes.
"""

import math
import sys

for _p in ("/opt/trn_rl_repo", "/opt/pypackages"):
    if _p not in sys.path:
        sys.path.append(_p)

import ml_dtypes
import numpy as np

import concourse.bacc as bacc
import concourse.bass as bass
import concourse.mybir as mybir
import concourse.tile as tile
from concourse.bass_utils import run_bass_kernel_spmd

F, M, C, OUT = 60, 600, 8, 18
B = 2048
NCORES = 8
BL = B // NCORES            # 256 batches per core
SB = 8                      # batches per superbatch
NSB = BL // SB              # 32 superbatches
FGB = 64                    # batches per finishing group
NFG = BL // FGB             # 4 finishing groups
SBPF = FGB // SB            # 8 superbatches per finishing group
CH = 5                      # chunks (of 120 rows) per batch
RP = M // CH                # 120 rows per chunk
XTW = 608                   # padded transposed row length
NG = FGB * C                # 512 finishing columns

BF16 = mybir.dt.bfloat16
F32 = mybir.dt.float32
MULT = mybir.AluOpType.mult
EPS = 1e-12

_CACHE = {}


def _build_nc():
    nc = bacc.Bacc(
        "TRN2", target_bir_lowering=False, debug=False,
        enable_asserts=False, num_devices=NCORES,
    )
    xg = nc.dram_tensor("xg", [NSB, RP, SB * CH * 64], BF16,
                        kind="ExternalInput").ap()
    xt = nc.dram_tensor("xt", [NSB // 2, 128, SB * XTW], BF16,
                        kind="ExternalInput").ap()
    waug_d = nc.dram_tensor("waug", [128, C], BF16, kind="ExternalInput").ap()
    cst_d = nc.dram_tensor("cst", [128, 112], F32, kind="ExternalInput").ap()
    hds_d = nc.dram_tensor("hds", [64, 2 * C * OUT], BF16,
                           kind="ExternalInput").ap()
    y = nc.dram_tensor("y", [NFG, OUT, FGB], F32, kind="ExternalOutput").ap()

    with tile.TileContext(nc) as tc:
        _emit(tc, y, xg, xt, waug_d, cst_d, hds_d)
    nc.compile()
    return nc


def _emit(tc, y, xg, xt, waug_d, cst_d, hds_d):
    nc = tc.nc
    from contextlib import ExitStack
    ctx = ExitStack()
    with ctx:
        cpool = ctx.enter_context(tc.tile_pool(name="cpool", bufs=1))
        xpool = ctx.enter_context(tc.tile_pool(name="xpool", bufs=8))
        qpool = ctx.enter_context(tc.tile_pool(name="qpool", bufs=4))
        tpool = ctx.enter_context(tc.tile_pool(name="tpool", bufs=4))
        spool = ctx.enter_context(tc.tile_pool(name="spool", bufs=3))
        gpool = ctx.enter_context(tc.tile_pool(name="gpool", bufs=2))
        fpool = ctx.enter_context(tc.tile_pool(name="fpool", bufs=1))
        lpsum = ctx.enter_context(tc.tile_pool(name="lpsum", bufs=2, space="PSUM"))
        fpsum = ctx.enter_context(tc.tile_pool(name="fpsum", bufs=3, space="PSUM"))
        bpsum = ctx.enter_context(tc.tile_pool(name="bpsum", bufs=1, space="PSUM"))
        rpsum = ctx.enter_context(tc.tile_pool(name="rpsum", bufs=2, space="PSUM"))

        # ---- constants ----
        waug = cpool.tile([128, C], BF16)
        nc.sync.dma_start(out=waug[:], in_=waug_d[:])
        cst = cpool.tile([128, 112], F32)
        nc.sync.dma_start(out=cst[:], in_=cst_d[:])
        hds = cpool.tile([64, 2 * C * OUT], BF16)
        nc.sync.dma_start(out=hds[:], in_=hds_d[:])
        # all f-indexed consts sit at rows 1:61 (f at row 1+f), except bcc
        # at rows 65:125; masks/ones/eps packed in spare cst columns
        k1 = cst[0:64, 0 * C:1 * C]
        w2k1 = cst[0:64, 1 * C:2 * C]
        bcc = cst[64:128, 2 * C:3 * C]
        cco = cst[0:64, 3 * C:4 * C]
        dco = cst[0:64, 4 * C:5 * C]
        maskA = cst[0:64, 40:41]           # rows 1:61 = 1 (partition-sum mask)
        eps1 = cst[0:1, 41:42]             # l2-norm epsilon (x s1^2)
        eps2 = cst[0:1, 42:43]             # l2-norm epsilon (x s2^2)
        ones_r = cst[0:1, 48:112]          # [1, 64] of ones (bcast lhsT)

        def cb(ap):  # broadcast a [64, C] const across FGB batches
            return ap.unsqueeze(1).broadcast_to([64, FGB, C])

        xtt_tiles = {}
        stage_tiles = {}
        fin_state = {}

        def prefetch(sb):
            if sb >= NSB:
                return
            t = sb // 2
            if sb % 2 == 0:
                xtt = tpool.tile([128, SB * XTW], BF16, name="xtt")
                nc.sync.dma_start(out=xtt[:], in_=xt[t])
                xtt_tiles[t] = xtt
            xgt = xpool.tile([RP, SB * CH * 64], BF16, tag="xgt", name="xgt")
            nc.sync.dma_start(out=xgt[:], in_=xg[sb])
            fin_state[("xgt", sb)] = xgt

        def emit_logits(sb):
            t = sb // 2
            po = 64 * (sb % 2)
            xtt = xtt_tiles[t]
            xgt = fin_state[("xgt", sb)]
            lp = lpsum.tile([128, SB * CH * C], F32)
            for b in range(SB):
                for c in range(CH):
                    nc.tensor.matmul(
                        lp[:, (b * CH + c) * C:(b * CH + c + 1) * C],
                        xtt[po:po + 61, b * XTW + c * RP: b * XTW + c * RP + 128],
                        waug[po:po + 61, :],
                        start=True, stop=True,
                    )
            # ---- softmax over C ----
            expt = spool.tile([RP, SB * CH * C], F32, tag="expt")
            nc.scalar.activation(
                expt[:], lp[0:RP, :], mybir.ActivationFunctionType.Exp
            )
            sums = spool.tile([RP, SB * CH], F32, tag="sums")
            nc.vector.reduce_sum(
                out=sums[:],
                in_=expt.rearrange("p (k e) -> p k e", e=C),
                axis=mybir.AxisListType.X,
            )
            rin = spool.tile([RP, SB * CH], F32, tag="rin")
            nc.vector.reciprocal(rin[:], sums[:])
            actt = spool.tile([RP, SB * CH * C], BF16, tag="actt")
            nc.vector.tensor_tensor(
                out=actt.rearrange("p (k e) -> p k e", e=C),
                in0=expt.rearrange("p (k e) -> p k e", e=C),
                in1=rin.unsqueeze(2).broadcast_to([RP, SB * CH, C]),
                op=MULT,
            )
            fin_state[("actt", sb)] = actt
            xsq = qpool.tile([RP, SB * CH * 64], BF16, tag="xsq", name="xsq")
            nc.scalar.square(xsq[:], xgt[:])
            fin_state[("xsq", sb)] = xsq

        def emit_fv(sb):
            fg, s = sb // SBPF, sb % SBPF
            if s == 0:
                stage_tiles[fg] = gpool.tile([128, NG], F32, tag="stage",
                                             name="stage")
            stage = stage_tiles[fg]
            xgt = fin_state.pop(("xgt", sb))
            xsq = fin_state.pop(("xsq", sb))
            actt = fin_state.pop(("actt", sb))
            fp = fpsum.tile([128, SB * C], F32)
            for b in range(SB):
                for c in range(CH):
                    k = b * CH + c
                    nc.tensor.matmul(
                        fp[0:64, b * C:(b + 1) * C],
                        xgt[:, k * 64:(k + 1) * 64],
                        actt[:, k * C:(k + 1) * C],
                        start=(c == 0), stop=(c == CH - 1),
                    )
                    nc.tensor.matmul(
                        fp[64:128, b * C:(b + 1) * C],
                        xsq[:, k * 64:(k + 1) * 64],
                        actt[:, k * C:(k + 1) * C],
                        start=(c == 0), stop=(c == CH - 1),
                        tile_position=(0, 64),
                    )
            nc.vector.tensor_copy(stage[:, s * SB * C:(s + 1) * SB * C], fp[:])

        # finishing, split into 4 stages emitted ~1 superbatch apart
        def emit_fin1(fg):
            stage = stage_tiles[fg]
            asb = bpsum.tile([64, NG], F32, tag="bcast")
            nc.tensor.matmul(asb[:], ones_r[:], stage[0:1, :],
                             start=True, stop=True)
            t1 = fpool.tile([64, NG], F32, tag="t1")
            nc.vector.tensor_tensor(out=t1.rearrange("p (g e) -> p g e", e=C),
                                    in0=stage[0:64, :].rearrange(
                                        "p (g e) -> p g e", e=C),
                                    in1=cb(k1), op=MULT)
            m1 = fpool.tile([64, NG], F32, tag="m1")
            nc.vector.tensor_tensor(out=m1.rearrange("p (g e) -> p g e", e=C),
                                    in0=asb.rearrange("p (g e) -> p g e", e=C),
                                    in1=cb(w2k1), op=MULT)
            fv1f = fpool.tile([64, NG], F32, tag="fv1f")
            nc.vector.tensor_sub(fv1f[:], t1[:], m1[:])
            q1 = fpool.tile([64, NG], F32, tag="q1")
            nc.vector.tensor_mul(q1[:], fv1f[:], fv1f[:])
            u1 = fpool.tile([64, NG], F32, tag="u1")
            nc.vector.tensor_tensor(out=u1.rearrange("p (g e) -> p g e", e=C),
                                    in0=asb.rearrange("p (g e) -> p g e", e=C),
                                    in1=cb(dco), op=MULT)
            u2 = fpool.tile([64, NG], F32, tag="u2")
            nc.vector.tensor_tensor(out=u2.rearrange("p (g e) -> p g e", e=C),
                                    in0=stage[64:128, :].rearrange(
                                        "p (g e) -> p g e", e=C),
                                    in1=cb(bcc), op=MULT)
            u3 = fpool.tile([64, NG], F32, tag="u3")
            nc.vector.tensor_add(u3[:], u1[:], u2[:])
            u4 = fpool.tile([64, NG], F32, tag="u4")
            nc.vector.tensor_tensor(out=u4.rearrange("p (g e) -> p g e", e=C),
                                    in0=stage[0:64, :].rearrange(
                                        "p (g e) -> p g e", e=C),
                                    in1=cb(cco), op=MULT)
            fv2n = fpool.tile([64, NG], F32, tag="fv2n")
            nc.vector.tensor_sub(fv2n[:], u3[:], u4[:])
            q2 = fpool.tile([64, NG], F32, tag="q2")
            nc.vector.tensor_mul(q2[:], fv2n[:], fv2n[:])
            fin_state[("f1", fg)] = (fv1f, q1, fv2n, q2)

        def emit_fin2(fg):
            fv1f, q1, fv2n, q2 = fin_state.pop(("f1", fg))
            r1 = rpsum.tile([1, NG], F32, tag="r")
            nc.tensor.matmul(r1[:], maskA[:], q1[:], start=True, stop=True)
            r2 = rpsum.tile([1, NG], F32, tag="r")
            nc.tensor.matmul(r2[:], maskA[:], q2[:], start=True, stop=True)
            r2c = fpool.tile([1, FGB], F32, tag="r2c")
            nc.vector.reduce_sum(out=r2c[:],
                                 in_=r2.rearrange("p (g e) -> p g e", e=C),
                                 axis=mybir.AxisListType.X)
            lr1 = fpool.tile([1, NG], F32, tag="lr1")
            nc.scalar.activation(lr1[:], r1[:],
                                 mybir.ActivationFunctionType.Ln, bias=eps1[:])
            lr2 = fpool.tile([1, FGB], F32, tag="lr2")
            nc.scalar.activation(lr2[:], r2c[:],
                                 mybir.ActivationFunctionType.Ln, bias=eps2[:])
            nr1 = fpool.tile([1, NG], F32, tag="nr1")
            nc.scalar.activation(nr1[:], lr1[:],
                                 mybir.ActivationFunctionType.Exp, scale=-0.5)
            nr2 = fpool.tile([1, FGB], F32, tag="nr2")
            nc.scalar.activation(nr2[:], lr2[:],
                                 mybir.ActivationFunctionType.Exp, scale=-0.5)
            nr2e = fpool.tile([1, NG], F32, tag="nr2e")
            nc.vector.tensor_copy(
                nr2e.rearrange("p (g e) -> p g e", e=C),
                nr2.unsqueeze(2).broadcast_to([1, FGB, C]),
            )
            fin_state[("f2", fg)] = (fv1f, fv2n, nr1, nr2e)

        def emit_fin3(fg):
            fv1f, fv2n, nr1, nr2e = fin_state.pop(("f2", fg))
            nb1 = bpsum.tile([64, NG], F32, tag="bcast")
            nc.tensor.matmul(nb1[:], ones_r[:], nr1[:], start=True, stop=True)
            fv1n = fpool.tile([64, NG], BF16, tag="fv1n")
            nc.vector.tensor_mul(fv1n[:], fv1f[:], nb1[:])
            nb2 = bpsum.tile([64, NG], F32, tag="bcast")
            nc.tensor.matmul(nb2[:], ones_r[:], nr2e[:], start=True, stop=True)
            fv2nn = fpool.tile([64, NG], BF16, tag="fv2nn")
            nc.vector.tensor_mul(fv2nn[:], fv2n[:], nb2[:])
            fin_state[("f3", fg)] = (fv1n, fv2nn)

        def emit_fin4(fg):
            fv1n, fv2nn = fin_state.pop(("f3", fg))
            hp = rpsum.tile([OUT, FGB], F32, tag="r")
            fv1v = fv1n.rearrange("p (g e) -> p g e", e=C)
            fv2v = fv2nn.rearrange("p (g e) -> p g e", e=C)
            for ci in range(C):
                nc.tensor.matmul(
                    hp[:], hds[:, ci * OUT:(ci + 1) * OUT], fv1v[:, :, ci],
                    start=(ci == 0), stop=False,
                )
            for ci in range(C):
                nc.tensor.matmul(
                    hp[:], hds[:, (C + ci) * OUT:(C + ci + 1) * OUT],
                    fv2v[:, :, ci],
                    start=False, stop=(ci == C - 1),
                )
            yt = fpool.tile([OUT, FGB], F32, tag="yt")
            nc.scalar.copy(yt[:], hp[:])
            nc.sync.dma_start(out=y[fg], in_=yt[:])

        fins = [emit_fin1, emit_fin2, emit_fin3, emit_fin4]

        # ---- software-pipelined main loop ----
        PF = 6
        for sb in range(PF):
            prefetch(sb)
        emit_logits(0)
        emit_logits(1)
        pend = []  # (fg, next_stage_idx)
        for sb in range(NSB):
            prefetch(sb + PF)
            if sb + 2 < NSB:
                emit_logits(sb + 2)
            emit_fv(sb)
            if sb % SBPF == SBPF - 1:
                pend.append([sb // SBPF, 0])
            # advance at most one finishing stage per superbatch
            if pend and pend[0][1] < 4 and sb >= 9:
                fg, st = pend[0]
                fins[st](fg)
                pend[0][1] += 1
                if pend[0][1] == 4:
                    pend.pop(0)
                    del stage_tiles[fg]
        # drain remaining finishing stages
        while pend:
            fg, st = pend[0]
            fins[st](fg)
            pend[0][1] += 1
            if pend[0][1] == 4:
                pend.pop(0)
                del stage_tiles[fg]


def _host_prep(reshaped_input, cluster_weights, covar_weights, cluster_biases,
               cluster_weights2, hidden1_weights):
    bf = ml_dtypes.bfloat16
    x = np.ascontiguousarray(reshaped_input, dtype=np.float32)
    xb = x.astype(bf)                                   # [B*M, F]

    # fv-side packed layout: [NCORES, NSB, RP, SB*CH, 64]; x^2 is
    # squared on-chip (DVE) from this same tile
    xgp = np.zeros((NCORES, NSB, RP, SB * CH, 64), dtype=bf)
    xgp[..., 0] = bf(1.0)
    xgp[..., 1:61] = (xb.reshape(NCORES, NSB, SB * CH, RP, F)
                        .transpose(0, 1, 3, 2, 4))
    xgp = xgp.reshape(NCORES, NSB, RP, SB * CH * 64)

    # logits-side transposed layout, two superbatches per 128 partitions
    x3 = xb.reshape(NCORES, NSB, SB, M, F)
    xtp = np.zeros((NCORES, NSB // 2, 2, 64, SB, XTW), dtype=bf)
    xtp[:, :, 0, 0:F, :, 0:M] = x3[:, 0::2].transpose(0, 1, 4, 2, 3)
    xtp[:, :, 1, 0:F, :, 0:M] = x3[:, 1::2].transpose(0, 1, 4, 2, 3)
    xtp[:, :, :, F, :, 0:M] = bf(1.0)
    xtp = xtp.reshape(NCORES, NSB // 2, 128, SB * XTW)

    waug = np.concatenate(
        [cluster_weights, cluster_biases[None, :]], axis=0
    ).astype(bf)                                        # [61, 8]
    waug2 = np.zeros((128, C), dtype=bf)
    waug2[0:61] = waug
    waug2[64:125] = waug

    cw = np.square(covar_weights.astype(np.float64)) + 1e-6       # [F, C]
    w2 = cluster_weights2[0].astype(np.float64)                   # [F, C]
    # per-chain scale factors keep the l2-norm sums in ACT-Ln's good
    # range (the Ln spline misbehaves above ~1e16); exactly cancelled by
    # the normalize when eps is scaled to match.
    s1 = 1.0 / float(np.abs(1.0 / cw).max())
    s2 = 1.0 / float(np.abs(1.0 / np.square(cw)).max())
    cst = np.zeros((128, 112), dtype=np.float32)
    cst[1:61, 0 * C:1 * C] = s1 / cw
    cst[1:61, 1 * C:2 * C] = s1 * w2 / cw
    cst[65:125, 2 * C:3 * C] = s2 / np.square(cw)
    cst[1:61, 3 * C:4 * C] = s2 * 2.0 * w2 / np.square(cw)
    cst[1:61, 4 * C:5 * C] = s2 * (np.square(w2) / np.square(cw) - 1.0)
    cst[1:61, 40] = 1.0          # maskA
    cst[0, 41] = EPS * s1 * s1   # eps1
    cst[0, 42] = EPS * s2 * s2   # eps2
    cst[0, 48:112] = 1.0         # ones_r

    h = hidden1_weights.astype(np.float64)              # [2*C*F, OUT]
    h1 = h[:C * F].reshape(F, C, OUT) / math.sqrt(C)    # fold 2nd l2n of fv1
    h2 = h[C * F:].reshape(F, C, OUT)
    hds = np.zeros((64, 2 * C * OUT), dtype=bf)
    hds[1:61] = np.concatenate([h1, h2], axis=1).reshape(F, 2 * C * OUT)

    in_maps = []
    for ci in range(NCORES):
        in_maps.append({
            "xg": np.ascontiguousarray(xgp[ci]),
            "xt": np.ascontiguousarray(xtp[ci]),
            "waug": waug2,
            "cst": cst,
            "hds": hds,
        })
    return in_maps


def _get_nc():
    if "nc" not in _CACHE:
        _CACHE["nc"] = _build_nc()
    return _CACHE["nc"]


def kernel(reshaped_input, cluster_weights, covar_weights, cluster_biases,
           cluster_weights2, hidden1_weights, **_kw):
    in_maps = _host_prep(reshaped_input, cluster_weights, covar_weights,
                         cluster_biases, cluster_weights2, hidden1_weights)
    nc = _get_nc()
    res = run_bass_kernel_spmd(nc, in_maps, list(range(NCORES)))
    ys = [
        res.results[ci]["y"].transpose(0, 2, 1).reshape(BL, OUT)
        for ci in range(NCORES)
    ]
    return np.ascontiguousarray(np.concatenate(ys, axis=0), dtype=np.float32)


if __name__ == "__main__":
    rng = np.random.default_rng(0)
    fake = {
        "reshaped_input": rng.standard_normal((B * M, F), dtype=np.float32),
        "cluster_weights": rng.standard_normal((F, C)).astype(np.float32) * 0.13,
        "covar_weights": rng.standard_normal((F, C)).astype(np.float32) * 0.13,
        "cluster_biases": rng.standard_normal((C,)).astype(np.float32) * 0.13,
        "cluster_weights2": rng.standard_normal((1, F, C)).astype(np.float32) * 0.13,
        "hidden1_weights": rng.standard_normal((2 * C * F, OUT)).astype(np.float32) * 0.35,
    }
    out = kernel(**fake)
    print("kernel output", out.shape, out.dtype, np.abs(out).mean())


# revision 13
# speedup vs baseline: 1.1540x; 1.0150x over previous
"""Trainium2 Bass kernel for nn_NetFV (NetFV pooling head).

Strategy (pure data parallel over 8 cores, 256 batches each):
  - Host: pack x into two bf16 layouts:
      xg [NSB, 120, SB*CH*128]: per 120-row chunk, cols 0=ones, 1:61=x,
        64:124=x^2 (fully contiguous per partition -> dense DMA packets;
        ones col makes the fv matmul emit asum for free at out row 0).
      xt2 [NSB/2, 128, SB*608]: transposed (f-major) x for the logits
        matmuls; TWO superbatches per tile at partition offsets 0 and 64
        so DMAs use (nearly) all 128 partitions.
  - Device, per superbatch of 8 batches (40 chunks of 120 rows):
      logits chunk [120,8] = matmul(lhsT=XT[61,128] (FWL), rhs=Waug[61,8])
      softmax: exp / rowsum / recip / mul over the whole superbatch
      fv chunk: matmul(lhsT=Xgrp[120,128]=(1|x|x^2) (FWL), rhs=act[120,8])
      -> psum [128(=asum|fv1|pad|fv2|pad), 8] per batch -> stage
  - Finishing per 64 batches, f-on-partitions, split into 4 pipeline
    stages interleaved with later superbatches so PE never stalls:
    elementwise DVE work as [60,512] ops with const broadcast APs;
    partition reductions/broadcasts via tiny PE matmuls with ones;
    1/sqrt via ACT Ln then Exp(scale=-0.5) (same ACT table set as the
    softmax Exp -> zero ACT table switches); head as 16 accumulated
    [60,18]^T x [60,64] matmuls into out.T [18,64]; host un-transposes.
"""

import math
import sys

for _p in ("/opt/trn_rl_repo", "/opt/pypackages"):
    if _p not in sys.path:
        sys.path.append(_p)

import ml_dtypes
import numpy as np

import concourse.bacc as bacc
import concourse.bass as bass
import concourse.mybir as mybir
import concourse.tile as tile
from concourse.bass_utils import run_bass_kernel_spmd

F, M, C, OUT = 60, 600, 8, 18
B = 2048
NCORES = 8
BL = B // NCORES            # 256 batches per core
SB = 8                      # batches per superbatch
NSB = BL // SB              # 32 superbatches
FGB = 64                    # batches per finishing group
NFG = BL // FGB             # 4 finishing groups
SBPF = FGB // SB            # 8 superbatches per finishing group
CH = 5                      # chunks (of 120 rows) per batch
RP = M // CH                # 120 rows per chunk
XTW = 608                   # padded transposed row length
NG = FGB * C                # 512 finishing columns

BF16 = mybir.dt.bfloat16
F32 = mybir.dt.float32
MULT = mybir.AluOpType.mult
EPS = 1e-12

_CACHE = {}


def _build_nc():
    nc = bacc.Bacc(
        "TRN2", target_bir_lowering=False, debug=False,
        enable_asserts=False, num_devices=NCORES,
    )
    xg = nc.dram_tensor("xg", [NSB, RP, SB * CH * 64], BF16,
                        kind="ExternalInput").ap()
    xt = nc.dram_tensor("xt", [NSB // 2, 128, SB * XTW], BF16,
                        kind="ExternalInput").ap()
    waug_d = nc.dram_tensor("waug", [128, C], BF16, kind="ExternalInput").ap()
    cst_d = nc.dram_tensor("cst", [128, 112], F32, kind="ExternalInput").ap()
    hds_d = nc.dram_tensor("hds", [64, 2 * C * OUT], BF16,
                           kind="ExternalInput").ap()
    y = nc.dram_tensor("y", [NFG, OUT, FGB], F32, kind="ExternalOutput").ap()

    with tile.TileContext(nc) as tc:
        _emit(tc, y, xg, xt, waug_d, cst_d, hds_d)
    nc.compile()
    return nc


def _emit(tc, y, xg, xt, waug_d, cst_d, hds_d):
    nc = tc.nc
    from contextlib import ExitStack
    ctx = ExitStack()
    with ctx:
        cpool = ctx.enter_context(tc.tile_pool(name="cpool", bufs=1))
        xpool = ctx.enter_context(tc.tile_pool(name="xpool", bufs=8))
        tpool = ctx.enter_context(tc.tile_pool(name="tpool", bufs=4))
        spool = ctx.enter_context(tc.tile_pool(name="spool", bufs=3))
        gpool = ctx.enter_context(tc.tile_pool(name="gpool", bufs=2))
        fpool = ctx.enter_context(tc.tile_pool(name="fpool", bufs=1))
        lpsum = ctx.enter_context(tc.tile_pool(name="lpsum", bufs=3, space="PSUM"))
        fpsum = ctx.enter_context(tc.tile_pool(name="fpsum", bufs=2, space="PSUM"))
        bpsum = ctx.enter_context(tc.tile_pool(name="bpsum", bufs=1, space="PSUM"))
        rpsum = ctx.enter_context(tc.tile_pool(name="rpsum", bufs=2, space="PSUM"))

        # ---- constants ----
        waug = cpool.tile([128, C], BF16)
        nc.sync.dma_start(out=waug[:], in_=waug_d[:])
        cst = cpool.tile([128, 112], F32)
        nc.sync.dma_start(out=cst[:], in_=cst_d[:])
        hds = cpool.tile([64, 2 * C * OUT], BF16)
        nc.sync.dma_start(out=hds[:], in_=hds_d[:])
        # all f-indexed consts sit at rows 1:61 (f at row 1+f), except bcc
        # at rows 65:125; masks/ones/eps packed in spare cst columns
        k1 = cst[0:64, 0 * C:1 * C]
        w2k1 = cst[0:64, 1 * C:2 * C]
        bcc = cst[64:128, 2 * C:3 * C]
        cco = cst[0:64, 3 * C:4 * C]
        dco = cst[0:64, 4 * C:5 * C]
        maskA = cst[0:64, 40:41]           # rows 1:61 = 1 (partition-sum mask)
        eps1 = cst[0:1, 41:42]             # l2-norm epsilon (x s1^2)
        eps2 = cst[0:1, 42:43]             # l2-norm epsilon (x s2^2)
        ones_r = cst[0:1, 48:112]          # [1, 64] of ones (bcast lhsT)

        def cb(ap):  # broadcast a [64, C] const across FGB batches
            return ap.unsqueeze(1).broadcast_to([64, FGB, C])

        xtt_tiles = {}
        stage_tiles = {}
        fin_state = {}

        def prefetch(sb):
            if sb >= NSB:
                return
            t = sb // 2
            if sb % 2 == 0:
                xtt = tpool.tile([128, SB * XTW], BF16, name="xtt")
                nc.sync.dma_start(out=xtt[:], in_=xt[t])
                xtt_tiles[t] = xtt
            xgt = xpool.tile([RP, SB * CH * 128], BF16, tag="xgt", name="xgt")
            nc.sync.dma_start(out=xgt[:, 0:SB * CH * 64], in_=xg[sb])
            fin_state[("xgt", sb)] = xgt

        def emit_logits(sb):
            t = sb // 2
            po = 64 * (sb % 2)
            xtt = xtt_tiles[t]
            xgt = fin_state[("xgt", sb)]
            lp = lpsum.tile([128, SB * CH * C], F32)
            for b in range(SB):
                for c in range(CH):
                    nc.tensor.matmul(
                        lp[:, (b * CH + c) * C:(b * CH + c + 1) * C],
                        xtt[po:po + 61, b * XTW + c * RP: b * XTW + c * RP + 128],
                        waug[po:po + 61, :],
                        start=True, stop=True,
                    )
            # ---- softmax over C ----
            expt = spool.tile([RP, SB * CH * C], F32, tag="expt")
            nc.scalar.activation(
                expt[:], lp[0:RP, :], mybir.ActivationFunctionType.Exp
            )
            sums = spool.tile([RP, SB * CH], F32, tag="sums")
            nc.vector.reduce_sum(
                out=sums[:],
                in_=expt.rearrange("p (k e) -> p k e", e=C),
                axis=mybir.AxisListType.X,
            )
            rin = spool.tile([RP, SB * CH], F32, tag="rin")
            nc.vector.reciprocal(rin[:], sums[:])
            actt = spool.tile([RP, SB * CH * C], BF16, tag="actt")
            nc.vector.tensor_tensor(
                out=actt.rearrange("p (k e) -> p k e", e=C),
                in0=expt.rearrange("p (k e) -> p k e", e=C),
                in1=rin.unsqueeze(2).broadcast_to([RP, SB * CH, C]),
                op=MULT,
            )
            fin_state[("actt", sb)] = actt
            nc.scalar.square(xgt[:, SB * CH * 64:], xgt[:, 0:SB * CH * 64])

        def emit_fv(sb):
            fg, s = sb // SBPF, sb % SBPF
            if s == 0:
                stage_tiles[fg] = gpool.tile([128, NG], F32, tag="stage",
                                             name="stage")
            stage = stage_tiles[fg]
            xgt = fin_state.pop(("xgt", sb))
            actt = fin_state.pop(("actt", sb))
            HB = SB * CH * 64
            fp = fpsum.tile([128, SB * C], F32)
            for b in range(SB):
                for c in range(CH):
                    k = b * CH + c
                    nc.tensor.matmul(
                        fp[0:64, b * C:(b + 1) * C],
                        xgt[:, k * 64:(k + 1) * 64],
                        actt[:, k * C:(k + 1) * C],
                        start=(c == 0), stop=(c == CH - 1),
                    )
                    nc.tensor.matmul(
                        fp[64:128, b * C:(b + 1) * C],
                        xgt[:, HB + k * 64:HB + (k + 1) * 64],
                        actt[:, k * C:(k + 1) * C],
                        start=(c == 0), stop=(c == CH - 1),
                        tile_position=(0, 64),
                    )
            nc.vector.tensor_copy(stage[:, s * SB * C:(s + 1) * SB * C], fp[:])

        # finishing, split into 4 stages emitted ~1 superbatch apart
        def emit_fin1(fg):
            stage = stage_tiles[fg]
            asb = bpsum.tile([64, NG], F32, tag="bcast")
            nc.tensor.matmul(asb[:], ones_r[:], stage[0:1, :],
                             start=True, stop=True)
            t1 = fpool.tile([64, NG], F32, tag="t1")
            nc.vector.tensor_tensor(out=t1.rearrange("p (g e) -> p g e", e=C),
                                    in0=stage[0:64, :].rearrange(
                                        "p (g e) -> p g e", e=C),
                                    in1=cb(k1), op=MULT)
            m1 = fpool.tile([64, NG], F32, tag="m1")
            nc.vector.tensor_tensor(out=m1.rearrange("p (g e) -> p g e", e=C),
                                    in0=asb.rearrange("p (g e) -> p g e", e=C),
                                    in1=cb(w2k1), op=MULT)
            fv1f = fpool.tile([64, NG], F32, tag="fv1f")
            nc.vector.tensor_sub(fv1f[:], t1[:], m1[:])
            q1 = fpool.tile([64, NG], F32, tag="q1")
            nc.vector.tensor_mul(q1[:], fv1f[:], fv1f[:])
            u1 = fpool.tile([64, NG], F32, tag="u1")
            nc.vector.tensor_tensor(out=u1.rearrange("p (g e) -> p g e", e=C),
                                    in0=asb.rearrange("p (g e) -> p g e", e=C),
                                    in1=cb(dco), op=MULT)
            u2 = fpool.tile([64, NG], F32, tag="u2")
            nc.vector.tensor_tensor(out=u2.rearrange("p (g e) -> p g e", e=C),
                                    in0=stage[64:128, :].rearrange(
                                        "p (g e) -> p g e", e=C),
                                    in1=cb(bcc), op=MULT)
            u3 = fpool.tile([64, NG], F32, tag="u3")
            nc.vector.tensor_add(u3[:], u1[:], u2[:])
            u4 = fpool.tile([64, NG], F32, tag="u4")
            nc.vector.tensor_tensor(out=u4.rearrange("p (g e) -> p g e", e=C),
                                    in0=stage[0:64, :].rearrange(
                                        "p (g e) -> p g e", e=C),
                                    in1=cb(cco), op=MULT)
            fv2n = fpool.tile([64, NG], F32, tag="fv2n")
            nc.vector.tensor_sub(fv2n[:], u3[:], u4[:])
            q2 = fpool.tile([64, NG], F32, tag="q2")
            nc.vector.tensor_mul(q2[:], fv2n[:], fv2n[:])
            fin_state[("f1", fg)] = (fv1f, q1, fv2n, q2)

        def emit_fin2(fg):
            fv1f, q1, fv2n, q2 = fin_state.pop(("f1", fg))
            r1 = rpsum.tile([1, NG], F32, tag="r")
            nc.tensor.matmul(r1[:], maskA[:], q1[:], start=True, stop=True)
            r2 = rpsum.tile([1, NG], F32, tag="r")
            nc.tensor.matmul(r2[:], maskA[:], q2[:], start=True, stop=True)
            r2c = fpool.tile([1, FGB], F32, tag="r2c")
            nc.vector.reduce_sum(out=r2c[:],
                                 in_=r2.rearrange("p (g e) -> p g e", e=C),
                                 axis=mybir.AxisListType.X)
            lr1 = fpool.tile([1, NG], F32, tag="lr1")
            nc.scalar.activation(lr1[:], r1[:],
                                 mybir.ActivationFunctionType.Ln, bias=eps1[:])
            lr2 = fpool.tile([1, FGB], F32, tag="lr2")
            nc.scalar.activation(lr2[:], r2c[:],
                                 mybir.ActivationFunctionType.Ln, bias=eps2[:])
            nr1 = fpool.tile([1, NG], F32, tag="nr1")
            nc.scalar.activation(nr1[:], lr1[:],
                                 mybir.ActivationFunctionType.Exp, scale=-0.5)
            nr2 = fpool.tile([1, FGB], F32, tag="nr2")
            nc.scalar.activation(nr2[:], lr2[:],
                                 mybir.ActivationFunctionType.Exp, scale=-0.5)
            nr2e = fpool.tile([1, NG], F32, tag="nr2e")
            nc.vector.tensor_copy(
                nr2e.rearrange("p (g e) -> p g e", e=C),
                nr2.unsqueeze(2).broadcast_to([1, FGB, C]),
            )
            fin_state[("f2", fg)] = (fv1f, fv2n, nr1, nr2e)

        def emit_fin3(fg):
            fv1f, fv2n, nr1, nr2e = fin_state.pop(("f2", fg))
            nb1 = bpsum.tile([64, NG], F32, tag="bcast")
            nc.tensor.matmul(nb1[:], ones_r[:], nr1[:], start=True, stop=True)
            fv1n = fpool.tile([64, NG], BF16, tag="fv1n")
            nc.vector.tensor_mul(fv1n[:], fv1f[:], nb1[:])
            nb2 = bpsum.tile([64, NG], F32, tag="bcast")
            nc.tensor.matmul(nb2[:], ones_r[:], nr2e[:], start=True, stop=True)
            fv2nn = fpool.tile([64, NG], BF16, tag="fv2nn")
            nc.vector.tensor_mul(fv2nn[:], fv2n[:], nb2[:])
            fin_state[("f3", fg)] = (fv1n, fv2nn)

        def emit_fin4(fg):
            fv1n, fv2nn = fin_state.pop(("f3", fg))
            hp = rpsum.tile([OUT, FGB], F32, tag="r")
            fv1v = fv1n.rearrange("p (g e) -> p g e", e=C)
            fv2v = fv2nn.rearrange("p (g e) -> p g e", e=C)
            for ci in range(C):
                nc.tensor.matmul(
                    hp[:], hds[:, ci * OUT:(ci + 1) * OUT], fv1v[:, :, ci],
                    start=(ci == 0), stop=False,
                )
            for ci in range(C):
                nc.tensor.matmul(
                    hp[:], hds[:, (C + ci) * OUT:(C + ci + 1) * OUT],
                    fv2v[:, :, ci],
                    start=False, stop=(ci == C - 1),
                )
            yt = fpool.tile([OUT, FGB], F32, tag="yt")
            nc.vector.tensor_copy(yt[:], hp[:])
            nc.sync.dma_start(out=y[fg], in_=yt[:])

        fins = [emit_fin1, emit_fin2, emit_fin3, emit_fin4]

        # ---- software-pipelined main loop ----
        PF = 6
        for sb in range(PF):
            prefetch(sb)
        emit_logits(0)
        emit_logits(1)
        pend = []  # (fg, next_stage_idx)
        for sb in range(NSB):
            prefetch(sb + PF)
            if sb + 2 < NSB:
                emit_logits(sb + 2)
            emit_fv(sb)
            if sb % SBPF == SBPF - 1:
                pend.append([sb // SBPF, 0])
            # advance at most one finishing stage per superbatch
            if pend and pend[0][1] < 4 and sb >= 9:
                fg, st = pend[0]
                fins[st](fg)
                pend[0][1] += 1
                if pend[0][1] == 4:
                    pend.pop(0)
                    del stage_tiles[fg]
        # drain remaining finishing stages
        while pend:
            fg, st = pend[0]
            fins[st](fg)
            pend[0][1] += 1
            if pend[0][1] == 4:
                pend.pop(0)
                del stage_tiles[fg]


def _host_prep(reshaped_input, cluster_weights, covar_weights, cluster_biases,
               cluster_weights2, hidden1_weights):
    bf = ml_dtypes.bfloat16
    x = np.ascontiguousarray(reshaped_input, dtype=np.float32)
    xb = x.astype(bf)                                   # [B*M, F]

    # fv-side packed layout: [NCORES, NSB, RP, SB*CH, 64]; x^2 is
    # squared on-chip (DVE) from this same tile
    xgp = np.zeros((NCORES, NSB, RP, SB * CH, 64), dtype=bf)
    xgp[..., 0] = bf(1.0)
    xgp[..., 1:61] = (xb.reshape(NCORES, NSB, SB * CH, RP, F)
                        .transpose(0, 1, 3, 2, 4))
    xgp = xgp.reshape(NCORES, NSB, RP, SB * CH * 64)

    # logits-side transposed layout, two superbatches per 128 partitions
    x3 = xb.reshape(NCORES, NSB, SB, M, F)
    xtp = np.zeros((NCORES, NSB // 2, 2, 64, SB, XTW), dtype=bf)
    xtp[:, :, 0, 0:F, :, 0:M] = x3[:, 0::2].transpose(0, 1, 4, 2, 3)
    xtp[:, :, 1, 0:F, :, 0:M] = x3[:, 1::2].transpose(0, 1, 4, 2, 3)
    xtp[:, :, :, F, :, 0:M] = bf(1.0)
    xtp = xtp.reshape(NCORES, NSB // 2, 128, SB * XTW)

    waug = np.concatenate(
        [cluster_weights, cluster_biases[None, :]], axis=0
    ).astype(bf)                                        # [61, 8]
    waug2 = np.zeros((128, C), dtype=bf)
    waug2[0:61] = waug
    waug2[64:125] = waug

    cw = np.square(covar_weights.astype(np.float64)) + 1e-6       # [F, C]
    w2 = cluster_weights2[0].astype(np.float64)                   # [F, C]
    # per-chain scale factors keep the l2-norm sums in ACT-Ln's good
    # range (the Ln spline misbehaves above ~1e16); exactly cancelled by
    # the normalize when eps is scaled to match.
    s1 = 1.0 / float(np.abs(1.0 / cw).max())
    s2 = 1.0 / float(np.abs(1.0 / np.square(cw)).max())
    cst = np.zeros((128, 112), dtype=np.float32)
    cst[1:61, 0 * C:1 * C] = s1 / cw
    cst[1:61, 1 * C:2 * C] = s1 * w2 / cw
    cst[65:125, 2 * C:3 * C] = s2 / np.square(cw)
    cst[1:61, 3 * C:4 * C] = s2 * 2.0 * w2 / np.square(cw)
    cst[1:61, 4 * C:5 * C] = s2 * (np.square(w2) / np.square(cw) - 1.0)
    cst[1:61, 40] = 1.0          # maskA
    cst[0, 41] = EPS * s1 * s1   # eps1
    cst[0, 42] = EPS * s2 * s2   # eps2
    cst[0, 48:112] = 1.0         # ones_r

    h = hidden1_weights.astype(np.float64)              # [2*C*F, OUT]
    h1 = h[:C * F].reshape(F, C, OUT) / math.sqrt(C)    # fold 2nd l2n of fv1
    h2 = h[C * F:].reshape(F, C, OUT)
    hds = np.zeros((64, 2 * C * OUT), dtype=bf)
    hds[1:61] = np.concatenate([h1, h2], axis=1).reshape(F, 2 * C * OUT)

    in_maps = []
    for ci in range(NCORES):
        in_maps.append({
            "xg": np.ascontiguousarray(xgp[ci]),
            "xt": np.ascontiguousarray(xtp[ci]),
            "waug": waug2,
            "cst": cst,
            "hds": hds,
        })
    return in_maps


def _get_nc():
    if "nc" not in _CACHE:
        _CACHE["nc"] = _build_nc()
    return _CACHE["nc"]


def kernel(reshaped_input, cluster_weights, covar_weights, cluster_biases,
           cluster_weights2, hidden1_weights, **_kw):
    in_maps = _host_prep(reshaped_input, cluster_weights, covar_weights,
                         cluster_biases, cluster_weights2, hidden1_weights)
    nc = _get_nc()
    res = run_bass_kernel_spmd(nc, in_maps, list(range(NCORES)))
    ys = [
        res.results[ci]["y"].transpose(0, 2, 1).reshape(BL, OUT)
        for ci in range(NCORES)
    ]
    return np.ascontiguousarray(np.concatenate(ys, axis=0), dtype=np.float32)


if __name__ == "__main__":
    rng = np.random.default_rng(0)
    fake = {
        "reshaped_input": rng.standard_normal((B * M, F), dtype=np.float32),
        "cluster_weights": rng.standard_normal((F, C)).astype(np.float32) * 0.13,
        "covar_weights": rng.standard_normal((F, C)).astype(np.float32) * 0.13,
        "cluster_biases": rng.standard_normal((C,)).astype(np.float32) * 0.13,
        "cluster_weights2": rng.standard_normal((1, F, C)).astype(np.float32) * 0.13,
        "hidden1_weights": rng.standard_normal((2 * C * F, OUT)).astype(np.float32) * 0.35,
    }
    out = kernel(**fake)
    print("kernel output", out.shape, out.dtype, np.abs(out).mean())


# revision 14
# speedup vs baseline: 1.1815x; 1.0238x over previous
"""Trainium2 Bass kernel for nn_NetFV (NetFV pooling head).

Strategy (pure data parallel over 8 cores, 256 batches each):
  - Host: pack x into two bf16 layouts:
      xg [NSB, 120, SB*CH*128]: per 120-row chunk, cols 0=ones, 1:61=x,
        64:124=x^2 (fully contiguous per partition -> dense DMA packets;
        ones col makes the fv matmul emit asum for free at out row 0).
      xt2 [NSB/2, 128, SB*608]: transposed (f-major) x for the logits
        matmuls; TWO superbatches per tile at partition offsets 0 and 64
        so DMAs use (nearly) all 128 partitions.
  - Device, per superbatch of 8 batches (40 chunks of 120 rows):
      logits chunk [120,8] = matmul(lhsT=XT[61,128] (FWL), rhs=Waug[61,8])
      softmax: exp / rowsum / recip / mul over the whole superbatch
      fv chunk: matmul(lhsT=Xgrp[120,128]=(1|x|x^2) (FWL), rhs=act[120,8])
      -> psum [128(=asum|fv1|pad|fv2|pad), 8] per batch -> stage
  - Finishing per 64 batches, f-on-partitions, split into 4 pipeline
    stages interleaved with later superbatches so PE never stalls:
    elementwise DVE work as [60,512] ops with const broadcast APs;
    partition reductions/broadcasts via tiny PE matmuls with ones;
    1/sqrt via ACT Ln then Exp(scale=-0.5) (same ACT table set as the
    softmax Exp -> zero ACT table switches); head as 16 accumulated
    [60,18]^T x [60,64] matmuls into out.T [18,64]; host un-transposes.
"""

import math
import sys

for _p in ("/opt/trn_rl_repo", "/opt/pypackages"):
    if _p not in sys.path:
        sys.path.append(_p)

import ml_dtypes
import numpy as np

import concourse.bacc as bacc
import concourse.bass as bass
import concourse.mybir as mybir
import concourse.tile as tile
from concourse.bass_utils import run_bass_kernel_spmd

F, M, C, OUT = 60, 600, 8, 18
B = 2048
NCORES = 8
BL = B // NCORES            # 256 batches per core
SB = 8                      # batches per superbatch
NSB = BL // SB              # 32 superbatches
FGB = 32                    # batches per finishing group
NFG = BL // FGB             # 4 finishing groups
SBPF = FGB // SB            # 8 superbatches per finishing group
CH = 5                      # chunks (of 120 rows) per batch
RP = M // CH                # 120 rows per chunk
XTW = 608                   # padded transposed row length
NG = FGB * C                # 512 finishing columns

BF16 = mybir.dt.bfloat16
F32 = mybir.dt.float32
MULT = mybir.AluOpType.mult
EPS = 1e-12

_CACHE = {}


def _build_nc():
    nc = bacc.Bacc(
        "TRN2", target_bir_lowering=False, debug=False,
        enable_asserts=False, num_devices=NCORES,
    )
    xg = nc.dram_tensor("xg", [NSB, RP, SB * CH * 64], BF16,
                        kind="ExternalInput").ap()
    xt = nc.dram_tensor("xt", [NSB // 2, 128, SB * XTW], BF16,
                        kind="ExternalInput").ap()
    waug_d = nc.dram_tensor("waug", [128, C], BF16, kind="ExternalInput").ap()
    cst_d = nc.dram_tensor("cst", [128, 112], F32, kind="ExternalInput").ap()
    hds_d = nc.dram_tensor("hds", [64, 2 * C * OUT], BF16,
                           kind="ExternalInput").ap()
    y = nc.dram_tensor("y", [NFG, OUT, FGB], F32, kind="ExternalOutput").ap()

    with tile.TileContext(nc) as tc:
        _emit(tc, y, xg, xt, waug_d, cst_d, hds_d)
    nc.compile()
    return nc


def _emit(tc, y, xg, xt, waug_d, cst_d, hds_d):
    nc = tc.nc
    from contextlib import ExitStack
    ctx = ExitStack()
    with ctx:
        cpool = ctx.enter_context(tc.tile_pool(name="cpool", bufs=1))
        xpool = ctx.enter_context(tc.tile_pool(name="xpool", bufs=8))
        tpool = ctx.enter_context(tc.tile_pool(name="tpool", bufs=4))
        spool = ctx.enter_context(tc.tile_pool(name="spool", bufs=3))
        gpool = ctx.enter_context(tc.tile_pool(name="gpool", bufs=2))
        fpool = ctx.enter_context(tc.tile_pool(name="fpool", bufs=1))
        lpsum = ctx.enter_context(tc.tile_pool(name="lpsum", bufs=3, space="PSUM"))
        fpsum = ctx.enter_context(tc.tile_pool(name="fpsum", bufs=2, space="PSUM"))
        bpsum = ctx.enter_context(tc.tile_pool(name="bpsum", bufs=1, space="PSUM"))
        rpsum = ctx.enter_context(tc.tile_pool(name="rpsum", bufs=2, space="PSUM"))

        # ---- constants ----
        waug = cpool.tile([128, C], BF16)
        nc.sync.dma_start(out=waug[:], in_=waug_d[:])
        cst = cpool.tile([128, 112], F32)
        nc.sync.dma_start(out=cst[:], in_=cst_d[:])
        hds = cpool.tile([64, 2 * C * OUT], BF16)
        nc.sync.dma_start(out=hds[:], in_=hds_d[:])
        # all f-indexed consts sit at rows 1:61 (f at row 1+f), except bcc
        # at rows 65:125; masks/ones/eps packed in spare cst columns
        k1 = cst[0:64, 0 * C:1 * C]
        w2k1 = cst[0:64, 1 * C:2 * C]
        bcc = cst[64:128, 2 * C:3 * C]
        cco = cst[0:64, 3 * C:4 * C]
        dco = cst[0:64, 4 * C:5 * C]
        maskA = cst[0:64, 40:41]           # rows 1:61 = 1 (partition-sum mask)
        eps1 = cst[0:1, 41:42]             # l2-norm epsilon (x s1^2)
        eps2 = cst[0:1, 42:43]             # l2-norm epsilon (x s2^2)
        ones_r = cst[0:1, 48:112]          # [1, 64] of ones (bcast lhsT)

        def cb(ap):  # broadcast a [64, C] const across FGB batches
            return ap.unsqueeze(1).broadcast_to([64, FGB, C])

        xtt_tiles = {}
        stage_tiles = {}
        fin_state = {}

        def prefetch(sb):
            if sb >= NSB:
                return
            t = sb // 2
            if sb % 2 == 0:
                xtt = tpool.tile([128, SB * XTW], BF16, name="xtt")
                nc.scalar.dma_start(out=xtt[:], in_=xt[t])
                xtt_tiles[t] = xtt
            xgt = xpool.tile([RP, SB * CH * 128], BF16, tag="xgt", name="xgt")
            nc.sync.dma_start(out=xgt[:, 0:SB * CH * 64], in_=xg[sb])
            fin_state[("xgt", sb)] = xgt

        def emit_logits(sb):
            t = sb // 2
            po = 64 * (sb % 2)
            xtt = xtt_tiles[t]
            xgt = fin_state[("xgt", sb)]
            lp = lpsum.tile([128, SB * CH * C], F32)
            for b in range(SB):
                for c in range(CH):
                    nc.tensor.matmul(
                        lp[:, (b * CH + c) * C:(b * CH + c + 1) * C],
                        xtt[po:po + 61, b * XTW + c * RP: b * XTW + c * RP + 128],
                        waug[po:po + 61, :],
                        start=True, stop=True,
                    )
            # ---- softmax over C (two halves for finer pipelining) ----
            expt = spool.tile([RP, SB * CH * C], F32, tag="expt")
            sums = spool.tile([RP, SB * CH], F32, tag="sums")
            rin = spool.tile([RP, SB * CH], F32, tag="rin")
            actt = spool.tile([RP, SB * CH * C], BF16, tag="actt")
            HW_ = SB * CH * C // 2
            HK = SB * CH // 2
            for h in range(2):
                ev = expt[:, h * HW_:(h + 1) * HW_].rearrange(
                    "p (k e) -> p k e", e=C)
                nc.scalar.activation(
                    expt[:, h * HW_:(h + 1) * HW_], lp[0:RP, h * HW_:(h + 1) * HW_],
                    mybir.ActivationFunctionType.Exp
                )
                nc.vector.reduce_sum(
                    out=sums[:, h * HK:(h + 1) * HK], in_=ev,
                    axis=mybir.AxisListType.X,
                )
                nc.vector.reciprocal(rin[:, h * HK:(h + 1) * HK],
                                     sums[:, h * HK:(h + 1) * HK])
                nc.vector.tensor_tensor(
                    out=actt[:, h * HW_:(h + 1) * HW_].rearrange(
                        "p (k e) -> p k e", e=C),
                    in0=ev,
                    in1=rin[:, h * HK:(h + 1) * HK].unsqueeze(2)
                        .broadcast_to([RP, HK, C]),
                    op=MULT,
                )
            fin_state[("actt", sb)] = actt
            nc.scalar.square(xgt[:, SB * CH * 64:], xgt[:, 0:SB * CH * 64])

        def emit_fv(sb):
            fg, s = sb // SBPF, sb % SBPF
            if s == 0:
                stage_tiles[fg] = gpool.tile([128, NG], F32, tag="stage",
                                             name="stage")
            stage = stage_tiles[fg]
            xgt = fin_state.pop(("xgt", sb))
            actt = fin_state.pop(("actt", sb))
            HB = SB * CH * 64
            fp = fpsum.tile([128, SB * C], F32)
            for b in range(SB):
                for c in range(CH):
                    k = b * CH + c
                    nc.tensor.matmul(
                        fp[0:64, b * C:(b + 1) * C],
                        xgt[:, k * 64:(k + 1) * 64],
                        actt[:, k * C:(k + 1) * C],
                        start=(c == 0), stop=(c == CH - 1),
                    )
                    nc.tensor.matmul(
                        fp[64:128, b * C:(b + 1) * C],
                        xgt[:, HB + k * 64:HB + (k + 1) * 64],
                        actt[:, k * C:(k + 1) * C],
                        start=(c == 0), stop=(c == CH - 1),
                        tile_position=(0, 64),
                    )
            nc.vector.tensor_copy(stage[:, s * SB * C:(s + 1) * SB * C], fp[:])

        # finishing, split into 4 stages emitted ~1 superbatch apart
        def emit_fin1(fg):
            stage = stage_tiles[fg]
            asb = bpsum.tile([64, NG], F32, tag="bcast")
            nc.tensor.matmul(asb[:], ones_r[:], stage[0:1, :],
                             start=True, stop=True)
            t1 = fpool.tile([64, NG], F32, tag="t1")
            nc.vector.tensor_tensor(out=t1.rearrange("p (g e) -> p g e", e=C),
                                    in0=stage[0:64, :].rearrange(
                                        "p (g e) -> p g e", e=C),
                                    in1=cb(k1), op=MULT)
            m1 = fpool.tile([64, NG], F32, tag="m1")
            nc.vector.tensor_tensor(out=m1.rearrange("p (g e) -> p g e", e=C),
                                    in0=asb.rearrange("p (g e) -> p g e", e=C),
                                    in1=cb(w2k1), op=MULT)
            fv1f = fpool.tile([64, NG], F32, tag="fv1f")
            nc.vector.tensor_sub(fv1f[:], t1[:], m1[:])
            q1 = fpool.tile([64, NG], F32, tag="q1")
            nc.vector.tensor_mul(q1[:], fv1f[:], fv1f[:])
            u1 = fpool.tile([64, NG], F32, tag="u1")
            nc.vector.tensor_tensor(out=u1.rearrange("p (g e) -> p g e", e=C),
                                    in0=asb.rearrange("p (g e) -> p g e", e=C),
                                    in1=cb(dco), op=MULT)
            u2 = fpool.tile([64, NG], F32, tag="u2")
            nc.vector.tensor_tensor(out=u2.rearrange("p (g e) -> p g e", e=C),
                                    in0=stage[64:128, :].rearrange(
                                        "p (g e) -> p g e", e=C),
                                    in1=cb(bcc), op=MULT)
            u3 = fpool.tile([64, NG], F32, tag="u3")
            nc.vector.tensor_add(u3[:], u1[:], u2[:])
            u4 = fpool.tile([64, NG], F32, tag="u4")
            nc.vector.tensor_tensor(out=u4.rearrange("p (g e) -> p g e", e=C),
                                    in0=stage[0:64, :].rearrange(
                                        "p (g e) -> p g e", e=C),
                                    in1=cb(cco), op=MULT)
            fv2n = fpool.tile([64, NG], F32, tag="fv2n")
            nc.vector.tensor_sub(fv2n[:], u3[:], u4[:])
            q2 = fpool.tile([64, NG], F32, tag="q2")
            nc.vector.tensor_mul(q2[:], fv2n[:], fv2n[:])
            fin_state[("f1", fg)] = (fv1f, q1, fv2n, q2)

        def emit_fin2(fg):
            fv1f, q1, fv2n, q2 = fin_state.pop(("f1", fg))
            r1 = rpsum.tile([1, NG], F32, tag="r")
            nc.tensor.matmul(r1[:], maskA[:], q1[:], start=True, stop=True)
            r2 = rpsum.tile([1, NG], F32, tag="r")
            nc.tensor.matmul(r2[:], maskA[:], q2[:], start=True, stop=True)
            r2c = fpool.tile([1, FGB], F32, tag="r2c")
            nc.vector.reduce_sum(out=r2c[:],
                                 in_=r2.rearrange("p (g e) -> p g e", e=C),
                                 axis=mybir.AxisListType.X)
            lr1 = fpool.tile([1, NG], F32, tag="lr1")
            nc.scalar.activation(lr1[:], r1[:],
                                 mybir.ActivationFunctionType.Ln, bias=eps1[:])
            lr2 = fpool.tile([1, FGB], F32, tag="lr2")
            nc.scalar.activation(lr2[:], r2c[:],
                                 mybir.ActivationFunctionType.Ln, bias=eps2[:])
            nr1 = fpool.tile([1, NG], F32, tag="nr1")
            nc.scalar.activation(nr1[:], lr1[:],
                                 mybir.ActivationFunctionType.Exp, scale=-0.5)
            nr2 = fpool.tile([1, FGB], F32, tag="nr2")
            nc.scalar.activation(nr2[:], lr2[:],
                                 mybir.ActivationFunctionType.Exp, scale=-0.5)
            nr2e = fpool.tile([1, NG], F32, tag="nr2e")
            nc.vector.tensor_copy(
                nr2e.rearrange("p (g e) -> p g e", e=C),
                nr2.unsqueeze(2).broadcast_to([1, FGB, C]),
            )
            fin_state[("f2", fg)] = (fv1f, fv2n, nr1, nr2e)

        def emit_fin3(fg):
            fv1f, fv2n, nr1, nr2e = fin_state.pop(("f2", fg))
            nb1 = bpsum.tile([64, NG], F32, tag="bcast")
            nc.tensor.matmul(nb1[:], ones_r[:], nr1[:], start=True, stop=True)
            fv1n = fpool.tile([64, NG], BF16, tag="fv1n")
            nc.vector.tensor_mul(fv1n[:], fv1f[:], nb1[:])
            nb2 = bpsum.tile([64, NG], F32, tag="bcast")
            nc.tensor.matmul(nb2[:], ones_r[:], nr2e[:], start=True, stop=True)
            fv2nn = fpool.tile([64, NG], BF16, tag="fv2nn")
            nc.vector.tensor_mul(fv2nn[:], fv2n[:], nb2[:])
            fin_state[("f3", fg)] = (fv1n, fv2nn)

        def emit_fin4(fg):
            fv1n, fv2nn = fin_state.pop(("f3", fg))
            hp = rpsum.tile([OUT, FGB], F32, tag="r")
            fv1v = fv1n.rearrange("p (g e) -> p g e", e=C)
            fv2v = fv2nn.rearrange("p (g e) -> p g e", e=C)
            for ci in range(C):
                nc.tensor.matmul(
                    hp[:], hds[:, ci * OUT:(ci + 1) * OUT], fv1v[:, :, ci],
                    start=(ci == 0), stop=False,
                )
            for ci in range(C):
                nc.tensor.matmul(
                    hp[:], hds[:, (C + ci) * OUT:(C + ci + 1) * OUT],
                    fv2v[:, :, ci],
                    start=False, stop=(ci == C - 1),
                )
            yt = fpool.tile([OUT, FGB], F32, tag="yt")
            nc.vector.tensor_copy(yt[:], hp[:])
            nc.sync.dma_start(out=y[fg], in_=yt[:])

        fins = [emit_fin1, emit_fin2, emit_fin3, emit_fin4]

        # ---- software-pipelined main loop ----
        PF = 6
        for sb in range(PF):
            prefetch(sb)
        emit_logits(0)
        emit_logits(1)
        pend = []  # (fg, next_stage_idx)
        for sb in range(NSB):
            prefetch(sb + PF)
            if sb + 2 < NSB:
                emit_logits(sb + 2)
            emit_fv(sb)
            if sb % SBPF == SBPF - 1:
                pend.append([sb // SBPF, 0])
            # advance at most one finishing stage per superbatch
            if pend and pend[0][1] < 4 and sb >= 4 * pend[0][0] + 4:
                fg, st = pend[0]
                fins[st](fg)
                pend[0][1] += 1
                if pend[0][1] == 4:
                    pend.pop(0)
                    del stage_tiles[fg]
        # drain remaining finishing stages
        while pend:
            fg, st = pend[0]
            fins[st](fg)
            pend[0][1] += 1
            if pend[0][1] == 4:
                pend.pop(0)
                del stage_tiles[fg]


def _host_prep(reshaped_input, cluster_weights, covar_weights, cluster_biases,
               cluster_weights2, hidden1_weights):
    bf = ml_dtypes.bfloat16
    x = np.ascontiguousarray(reshaped_input, dtype=np.float32)
    xb = x.astype(bf)                                   # [B*M, F]

    # fv-side packed layout: [NCORES, NSB, RP, SB*CH, 64]; x^2 is
    # squared on-chip (DVE) from this same tile
    xgp = np.zeros((NCORES, NSB, RP, SB * CH, 64), dtype=bf)
    xgp[..., 0] = bf(1.0)
    xgp[..., 1:61] = (xb.reshape(NCORES, NSB, SB * CH, RP, F)
                        .transpose(0, 1, 3, 2, 4))
    xgp = xgp.reshape(NCORES, NSB, RP, SB * CH * 64)

    # logits-side transposed layout, two superbatches per 128 partitions
    x3 = xb.reshape(NCORES, NSB, SB, M, F)
    xtp = np.zeros((NCORES, NSB // 2, 2, 64, SB, XTW), dtype=bf)
    xtp[:, :, 0, 0:F, :, 0:M] = x3[:, 0::2].transpose(0, 1, 4, 2, 3)
    xtp[:, :, 1, 0:F, :, 0:M] = x3[:, 1::2].transpose(0, 1, 4, 2, 3)
    xtp[:, :, :, F, :, 0:M] = bf(1.0)
    xtp = xtp.reshape(NCORES, NSB // 2, 128, SB * XTW)

    waug = np.concatenate(
        [cluster_weights, cluster_biases[None, :]], axis=0
    ).astype(bf)                                        # [61, 8]
    waug2 = np.zeros((128, C), dtype=bf)
    waug2[0:61] = waug
    waug2[64:125] = waug

    cw = np.square(covar_weights.astype(np.float64)) + 1e-6       # [F, C]
    w2 = cluster_weights2[0].astype(np.float64)                   # [F, C]
    # per-chain scale factors keep the l2-norm sums in ACT-Ln's good
    # range (the Ln spline misbehaves above ~1e16); exactly cancelled by
    # the normalize when eps is scaled to match.
    s1 = 1.0 / float(np.abs(1.0 / cw).max())
    s2 = 1.0 / float(np.abs(1.0 / np.square(cw)).max())
    cst = np.zeros((128, 112), dtype=np.float32)
    cst[1:61, 0 * C:1 * C] = s1 / cw
    cst[1:61, 1 * C:2 * C] = s1 * w2 / cw
    cst[65:125, 2 * C:3 * C] = s2 / np.square(cw)
    cst[1:61, 3 * C:4 * C] = s2 * 2.0 * w2 / np.square(cw)
    cst[1:61, 4 * C:5 * C] = s2 * (np.square(w2) / np.square(cw) - 1.0)
    cst[1:61, 40] = 1.0          # maskA
    cst[0, 41] = EPS * s1 * s1   # eps1
    cst[0, 42] = EPS * s2 * s2   # eps2
    cst[0, 48:112] = 1.0         # ones_r

    h = hidden1_weights.astype(np.float64)              # [2*C*F, OUT]
    h1 = h[:C * F].reshape(F, C, OUT) / math.sqrt(C)    # fold 2nd l2n of fv1
    h2 = h[C * F:].reshape(F, C, OUT)
    hds = np.zeros((64, 2 * C * OUT), dtype=bf)
    hds[1:61] = np.concatenate([h1, h2], axis=1).reshape(F, 2 * C * OUT)

    in_maps = []
    for ci in range(NCORES):
        in_maps.append({
            "xg": np.ascontiguousarray(xgp[ci]),
            "xt": np.ascontiguousarray(xtp[ci]),
            "waug": waug2,
            "cst": cst,
            "hds": hds,
        })
    return in_maps


def _get_nc():
    if "nc" not in _CACHE:
        _CACHE["nc"] = _build_nc()
    return _CACHE["nc"]


def kernel(reshaped_input, cluster_weights, covar_weights, cluster_biases,
           cluster_weights2, hidden1_weights, **_kw):
    in_maps = _host_prep(reshaped_input, cluster_weights, covar_weights,
                         cluster_biases, cluster_weights2, hidden1_weights)
    nc = _get_nc()
    res = run_bass_kernel_spmd(nc, in_maps, list(range(NCORES)))
    ys = [
        res.results[ci]["y"].transpose(0, 2, 1).reshape(BL, OUT)
        for ci in range(NCORES)
    ]
    return np.ascontiguousarray(np.concatenate(ys, axis=0), dtype=np.float32)


if __name__ == "__main__":
    rng = np.random.default_rng(0)
    fake = {
        "reshaped_input": rng.standard_normal((B * M, F), dtype=np.float32),
        "cluster_weights": rng.standard_normal((F, C)).astype(np.float32) * 0.13,
        "covar_weights": rng.standard_normal((F, C)).astype(np.float32) * 0.13,
        "cluster_biases": rng.standard_normal((C,)).astype(np.float32) * 0.13,
        "cluster_weights2": rng.standard_normal((1, F, C)).astype(np.float32) * 0.13,
        "hidden1_weights": rng.standard_normal((2 * C * F, OUT)).astype(np.float32) * 0.35,
    }
    out = kernel(**fake)
    print("kernel output", out.shape, out.dtype, np.abs(out).mean())
